# revision 11
# baseline (speedup 1.0000x reference)
"""Multi-head self-attention Trainium2 Bass kernel (8-core SPMD).

Sharding: data-parallel over query rows. The flattened (B*S, D) = (8192, 512)
query space is split into 8 blocks of 1024 rows; core c handles batch c//4,
query rows (c%4)*1024 .. +1024. Each core recomputes K/V for its whole batch
(4-way duplicated) which avoids any cross-core communication; host-side
gather is a pure concatenation.

Layout strategy: activations live transposed in SBUF ([D, S], d on
partitions). Projections then need no weight transposes:
  K^T = Wk^T x^T   (lhsT = Wk chunk, rhs = x^T chunk)
  V   = x Wv       (lhsT = x^T chunk, rhs = Wv chunk)
Scores are computed transposed ([k, q], k on partitions) so softmax's
denominator comes from a ones-column appended to V (row 64 of the attention
output accumulator), and A^T is directly consumable by the A@V matmul.
exp() runs on the scalar engine with the 1/sqrt(dk) folded into its scale.
The normalized per-head outputs O^T are exactly the lhsT the output
projection wants, so no transposes are needed anywhere except on the input x.

Matmul operands are stored as fp16 (10-bit mantissa; measured end-to-end
absmax relative error ~4e-4): unlike f32r this uses the true MAC path, so
the PE clock-gate (HAM) warms to 2.4 GHz and fast weight load applies.
All accumulation is fp32 in PSUM; softmax denominators/reciprocals are fp32.

PSUM budget (8 banks): one shared pool of [128,512] tiles (bufs=6) serves
transposes, projections, scores, the denominator broadcast and the output
projection; a 2-buffer pool holds the per-head attention accumulators.
"""

from contextlib import ExitStack

import numpy as np

import concourse.bass as bass
import concourse.tile as tile
from concourse import bacc, mybir
from concourse.bass_utils import run_bass_kernel_spmd

N_CORES = 8
B, S, D, H, DK = 2, 4096, 512, 8, 64
QL = B * S // N_CORES          # 1024 query rows per core
P = 128
NT_S = S // P                  # 32 sequence tiles
NT_D = D // P                  # 4 d-model chunks
QC = QL // 512                 # 2 query chunks of 512
F32 = mybir.dt.float32
F32R = mybir.dt.float32r
F16 = mybir.dt.float16
EXP = mybir.ActivationFunctionType.Exp

# "f16" (10 mantissa bits, 2.4 GHz MAC path + FWL), "f32r" (13 bits but
# pinned at the 1.2 GHz throttled clock), "f32" (exact, 4 cycles/row).
MM_DTYPE = "f16"
DTM = {"f32r": F32R, "f16": F16, "f32": F32}[MM_DTYPE]


def _emit(ctx: ExitStack, tc: tile.TileContext, io: dict):
    nc = tc.nc
    xb, xq = io["xb"], io["xq"]
    wq, wk, wv, wo = io["wq"], io["wk"], io["wv"], io["wo"]
    bq, bk, bv, bo = io["bq"], io["bk"], io["bv"], io["bo"]
    ident = io["ident"]
    out = io["out"]

    mm = nc.tensor.matmul

    # ---- pools persistent across the whole kernel ------------------------
    consts = ctx.enter_context(tc.tile_pool(name="consts", bufs=1))
    xt_pool = ctx.enter_context(tc.tile_pool(name="xt", bufs=1))
    qt_pool = ctx.enter_context(tc.tile_pool(name="qt", bufs=4))
    # PSUM: shared [128,512] pool (6 banks) + attention accumulators (2).
    ps_pool = ctx.enter_context(tc.tile_pool(name="ps", bufs=6, space="PSUM"))
    o_pool = ctx.enter_context(tc.tile_pool(name="o", bufs=2, space="PSUM"))

    def psum512():
        return ps_pool.tile([P, 512], F32, tag="ps", name="ps")

    # ---- constants --------------------------------------------------------
    ident_f32 = consts.tile([P, P], F32, tag="ident32")
    nc.sync.dma_start(out=ident_f32[:], in_=ident[:])
    ident_sb = consts.tile([P, P], DTM, tag="ident")
    nc.vector.tensor_copy(out=ident_sb[:], in_=ident_f32[:])
    ones_f32 = consts.tile([P, 1], F32, tag="ones_f32")
    nc.vector.memset(ones_f32[:], 1.0)
    ones_sb = consts.tile([1, 512], DTM, tag="ones")
    nc.vector.tensor_copy(out=ones_sb[:], in_=ones_f32[0:1, 0:1].broadcast_to([1, 512]))
    # a f32 ones row living on partition 64 (denominator broadcast lhsT)
    ones64_sb = consts.tile([65, 64], F32, tag="ones64")
    nc.vector.memset(ones64_sb[64:65, :], 1.0)
    bias_sb = {}
    with tc.tile_pool(name="stg0", bufs=2) as stg0:
        for nm, ap in (("bq", bq), ("bk", bk), ("bv", bv), ("bo", bo)):
            st = stg0.tile([1, D], F32, tag="bstg")
            nc.sync.dma_start(out=st[:], in_=ap[:])
            t = consts.tile([1, D], DTM, tag=nm)
            nc.vector.tensor_copy(out=t[:], in_=st[:])
            bias_sb[nm] = t

    xT = xt_pool.tile([P, NT_D * S], DTM, tag="xT")
    wq_r = wq.rearrange("(dc p) m -> p dc m", p=P)
    wk_r = wk.rearrange("(dc p) m -> p dc m", p=P)
    wv_r = wv.rearrange("(dc p) m -> p dc m", p=P)
    QT = []

    with tc.tile_pool(name="xq", bufs=1) as xq_pool:
        xqT = xq_pool.tile([P, NT_D * QL], DTM, tag="xqT")

        # ---- stage A: x^T and xq^T via PE transposes ---------------------
        with tc.tile_pool(name="xn", bufs=3) as xn_pool:
            for src_ap, ntile, dst in ((xb, NT_S, xT), (xq, QL // P, xqT)):
                for st in range(ntile):
                    xn = xn_pool.tile([P, D], F32, tag="xn")
                    nc.sync.dma_start(out=xn[:], in_=src_ap[st * P:(st + 1) * P, :])
                    xh = xn_pool.tile([P, D], DTM, tag="xh")
                    nc.vector.tensor_copy(out=xh[:], in_=xn[:])
                    tp = ps_pool.tile([P, 512], DTM, tag="ps", name="tp")
                    for dc in range(NT_D):
                        nc.tensor.transpose(
                            tp[:, dc * P:(dc + 1) * P],
                            xh[:, dc * P:(dc + 1) * P],
                            ident_sb[:],
                        )
                    dst_ap = dst[:, :].rearrange("p (dc s) -> p dc s", dc=NT_D)
                    nc.vector.tensor_copy(
                        out=dst_ap[:, :, st * P:(st + 1) * P],
                        in_=tp[:, :].rearrange("p (dc j) -> p dc j", dc=NT_D),
                    )

        # ---- stage B: Q^T for all 4 head pairs ---------------------------
        with (
            tc.tile_pool(name="wqp", bufs=1) as wq_pool,
            tc.tile_pool(name="stgb", bufs=2) as stgb,
        ):
            for pr in range(4):
                wst = stgb.tile([P, NT_D * P], F32, tag="wstg")
                nc.sync.dma_start(
                    out=wst[:, :].rearrange("p (dc m) -> p dc m", dc=NT_D),
                    in_=wq_r[:, :, pr * P:(pr + 1) * P],
                )
                wqp = wq_pool.tile([P, NT_D * P], DTM, tag="wq")
                nc.vector.tensor_copy(out=wqp[:], in_=wst[:])
                qt = qt_pool.tile([P, QL], DTM, tag="QT")
                for qc in range(QC):
                    ps = psum512()
                    for dc in range(NT_D):
                        mm(ps[:], wqp[:, dc * P:(dc + 1) * P],
                           xqT[:, dc * QL + qc * 512:dc * QL + (qc + 1) * 512],
                           start=(dc == 0), stop=False)
                    mm(ps[:], bias_sb["bq"][0:1, pr * P:(pr + 1) * P],
                       ones_sb[0:1, :], start=False, stop=True)
                    nc.vector.tensor_copy(out=qt[:, qc * 512:(qc + 1) * 512],
                                          in_=ps[:])
                QT.append(qt)

    # ---- stage C: per 4-head group: V, then per pair K^T + attention -----
    OT = []  # per-head [64, QL] normalized attention outputs (transposed)
    with tc.tile_pool(name="ot", bufs=8) as ot_pool:
        with (
            tc.tile_pool(name="wkv", bufs=1) as wkv_pool,
            tc.tile_pool(name="stgc", bufs=1) as stgc,
            tc.tile_pool(name="kt", bufs=1) as kt_pool,
            tc.tile_pool(name="v", bufs=1) as v_pool,
            tc.tile_pool(name="e", bufs=6) as e_pool,
            tc.tile_pool(name="rc", bufs=4) as rc_pool,
        ):
            for g in range(2):
                # V for the group's 4 heads, augmented with a ones column:
                # vaug[:, st*260 + hl*65 + (0..63)] = V[st block, head hl]
                # vaug[:, st*260 + hl*65 + 64]      = 1.0
                wst = stgc.tile([P, NT_D * 256], F32, tag="wstg")
                nc.sync.dma_start(
                    out=wst[:, :].rearrange("p (dc m) -> p dc m", dc=NT_D),
                    in_=wv_r[:, :, g * 256:(g + 1) * 256],
                )
                wvg = wkv_pool.tile([P, NT_D * 256], DTM, tag="wv")
                nc.vector.tensor_copy(out=wvg[:], in_=wst[:])
                vaug = v_pool.tile([P, NT_S * 260], DTM, tag="vaug")
                nc.vector.tensor_copy(
                    out=vaug[:, :].rearrange("p (t h e) -> p t h e",
                                             t=NT_S, h=4)[:, :, :, 64:65],
                    in_=ones_f32[:, 0:1].broadcast_to([P, NT_S, 4, 1]),
                )
                for st in range(NT_S):
                    ps = psum512()
                    for dc in range(NT_D):
                        mm(ps[:, 0:256],
                           xT[:, dc * S + st * P:dc * S + (st + 1) * P],
                           wvg[:, dc * 256:(dc + 1) * 256],
                           start=(dc == 0), stop=False)
                    mm(ps[:, 0:256], ones_sb[0:1, 0:P],
                       bias_sb["bv"][0:1, g * 256:(g + 1) * 256],
                       start=False, stop=True)
                    dst = vaug[:, st * 260:(st + 1) * 260]
                    dst = dst.rearrange("p (h e) -> p h e", h=4)[:, :, 0:64]
                    nc.vector.tensor_copy(
                        out=dst,
                        in_=ps[:, 0:256].rearrange("p (h e) -> p h e", h=4),
                    )

                for pi in range(2):
                    pr = 2 * g + pi
                    wst = stgc.tile([P, NT_D * 256], F32, tag="wstg")
                    nc.sync.dma_start(
                        out=wst[:, 0:NT_D * P].rearrange("p (dc m) -> p dc m",
                                                         dc=NT_D),
                        in_=wk_r[:, :, pr * P:(pr + 1) * P],
                    )
                    wkp = wkv_pool.tile([P, NT_D * P], DTM, tag="wk")
                    nc.vector.tensor_copy(out=wkp[:], in_=wst[:, 0:NT_D * P])
                    kt = kt_pool.tile([P, S], DTM, tag="KT")
                    for sc in range(8):
                        ps = psum512()
                        for dc in range(NT_D):
                            mm(ps[:], wkp[:, dc * P:(dc + 1) * P],
                               xT[:, dc * S + sc * 512:dc * S + (sc + 1) * 512],
                               start=(dc == 0), stop=False)
                        mm(ps[:], bias_sb["bk"][0:1, pr * P:(pr + 1) * P],
                           ones_sb[0:1, :], start=False, stop=True)
                        nc.vector.tensor_copy(out=kt[:, sc * 512:(sc + 1) * 512],
                                              in_=ps[:])

                    ot0 = ot_pool.tile([64, QL], DTM, tag="OT")
                    ot1 = ot_pool.tile([64, QL], DTM, tag="OT")
                    OT += [ot0, ot1]
                    qt = QT[pr]
                    hl0, hl1 = 2 * pi, 2 * pi + 1
                    for qc in range(QC):
                        qsl = slice(qc * 512, (qc + 1) * 512)
                        o0 = o_pool.tile([65, 512], F32, tag="O")
                        o1 = o_pool.tile([65, 512], F32, tag="O")
                        for ktile in range(NT_S):
                            ksl = slice(ktile * P, (ktile + 1) * P)
                            fl = dict(start=(ktile == 0),
                                      stop=(ktile == NT_S - 1))
                            st_ = ktile * 260
                            # heads ride PE row strips 0-63 / 64-127
                            sp0 = psum512()
                            sp1 = psum512()
                            # keep the pair adjacent in the PE stream: the
                            # h1 weight load targets rows 64-127 and pulls
                            # ahead of the in-flight h0 matmul (rows 0-63),
                            # so the two heads stream concurrently.
                            with tc.tile_critical():
                                mm(sp0[:], kt[0:64, ksl], qt[0:64, qsl])
                                mm(sp1[:], kt[64:128, ksl], qt[64:128, qsl])
                            ea0 = e_pool.tile([P, 512], DTM, tag="ea")
                            ea1 = e_pool.tile([P, 512], DTM, tag="ea")
                            nc.scalar.activation(ea0[:], sp0[:], EXP, scale=0.125)
                            nc.scalar.activation(ea1[:], sp1[:], EXP, scale=0.125)
                            mm(o0[:], vaug[:, st_ + hl0 * 65:st_ + hl0 * 65 + 65],
                               ea0[:], **fl)
                            mm(o1[:], vaug[:, st_ + hl1 * 65:st_ + hl1 * 65 + 65],
                               ea1[:], **fl)
                        # normalize: O[0:64] * (1 / O[64]) broadcast down.
                        # Copy O out of PSUM immediately (frees the bank),
                        # then run the denominator chain out of SBUF.
                        for o_ps, ot in ((o0, ot0), (o1, ot1)):
                            osb = rc_pool.tile([65, 512], F32, tag="osb")
                            nc.vector.tensor_copy(out=osb[:], in_=o_ps[:])
                            bc = psum512()
                            mm(bc[0:64, :], ones64_sb[64:65, :], osb[64:65, :])
                            rbc = rc_pool.tile([64, 512], F32, tag="rbc")
                            nc.vector.reciprocal(out=rbc[:], in_=bc[0:64, :])
                            nc.vector.tensor_mul(ot[:, qsl], osb[0:64, :], rbc[:])

        # ---- stage D: output projection Y = concat_h(O_h) @ Wo + bo ------
        with (
            tc.tile_pool(name="wo", bufs=8) as wo_pool,
            tc.tile_pool(name="y", bufs=2) as y_pool,
        ):
            wo_sb = []
            for h in range(H):
                wst = y_pool.tile([64, D], F32, tag="wostg")
                nc.sync.dma_start(out=wst[:], in_=wo[h * 64:(h + 1) * 64, :])
                woh = wo_pool.tile([64, D], DTM, tag="wo")
                nc.vector.tensor_copy(out=woh[:], in_=wst[:])
                wo_sb.append(woh)
            for qt_i in range(QL // P):
                ps = psum512()
                for h in range(H):
                    mm(ps[:], OT[h][:, qt_i * P:(qt_i + 1) * P], wo_sb[h][:],
                       start=(h == 0), stop=False)
                mm(ps[:], ones_sb[0:1, 0:P], bias_sb["bo"][0:1, :],
                   start=False, stop=True)
                ysb = y_pool.tile([P, D], F32, tag="y")
                nc.vector.tensor_copy(out=ysb[:], in_=ps[:])
                nc.sync.dma_start(out=out[qt_i * P:(qt_i + 1) * P, :], in_=ysb[:])


def build():
    nc = bacc.Bacc("TRN2", target_bir_lowering=False, debug=False,
                   num_devices=N_CORES)
    io = {}
    for nm, shape in (("xb", [S, D]), ("xq", [QL, D]), ("wq", [D, D]),
                      ("wk", [D, D]), ("wv", [D, D]), ("wo", [D, D]),
                      ("bq", [1, D]), ("bk", [1, D]), ("bv", [1, D]),
                      ("bo", [1, D]), ("ident", [P, P])):
        io[nm] = nc.dram_tensor(nm, shape, F32, kind="ExternalInput").ap()
    io["out"] = nc.dram_tensor("out", [QL, D], F32, kind="ExternalOutput").ap()
    with tile.TileContext(nc) as tc:
        with ExitStack() as ctx:
            _emit(ctx, tc, io)
    nc.compile()
    return nc


def make_in_maps(inputs):
    f = lambda a: np.ascontiguousarray(np.asarray(a, dtype=np.float32))
    x = f(inputs["x"])
    fixed = {
        "wq": f(inputs["Wq"]), "wk": f(inputs["Wk"]), "wv": f(inputs["Wv"]),
        "wo": f(inputs["Wo"]),
        "bq": f(inputs["bq"]).reshape(1, D), "bk": f(inputs["bk"]).reshape(1, D),
        "bv": f(inputs["bv"]).reshape(1, D), "bo": f(inputs["bo"]).reshape(1, D),
        "ident": np.eye(P, dtype=np.float32),
    }
    in_maps = []
    for c in range(N_CORES):
        b, qs = c // 4, (c % 4) * QL
        in_maps.append({"xb": x[b], "xq": x[b, qs:qs + QL], **fixed})
    return in_maps


_CACHE = {}
LAST_EXEC_NS = None


def run(inputs, trace=False):
    global LAST_EXEC_NS
    if "nc" not in _CACHE:
        _CACHE["nc"] = build()
    nc = _CACHE["nc"]
    kw = {}
    if trace:
        import sys, types
        if "antenv.axon_hooks" not in sys.modules:
            sys.path.insert(0, "/root/.axon_site")
            try:
                from trn_agent_boot.trn_boot import _ntff_profile_via_ctypes
                hook = _ntff_profile_via_ctypes("/opt/axon/libaxon_pjrt.so")
                mod = types.ModuleType("antenv.axon_hooks")
                mod.get_axon_ntff_profile_hook = lambda: hook
                mod.set_axon_ntff_profile_hook = lambda h: None
                sys.modules["antenv.axon_hooks"] = mod
            except Exception:
                pass
        kw = dict(trace=True, trace_cores=[0])
    res = run_bass_kernel_spmd(nc, make_in_maps(inputs),
                               core_ids=list(range(N_CORES)), **kw)
    if trace:
        LAST_EXEC_NS = res.exec_time_ns
    out = np.empty((B, S, D), np.float32)
    for c in range(N_CORES):
        b, qs = c // 4, (c % 4) * QL
        out[b, qs:qs + QL] = res.results[c]["out"]
    return out


def kernel(**inputs) -> np.ndarray:
    return run(inputs, trace=False)


# revision 14
# speedup vs baseline: 1.2744x; 1.2744x over previous
"""Multi-head self-attention Trainium2 Bass kernel (8-core SPMD).

Sharding: data-parallel over query rows. The flattened (B*S, D) = (8192, 512)
query space is split into 8 blocks of 1024 rows; core c handles batch c//4,
query rows (c%4)*1024 .. +1024. Each core recomputes K/V for its whole batch
(4-way duplicated) which avoids any cross-core communication; host-side
gather is a pure concatenation.

Layout strategy: activations live transposed in SBUF ([D, S], d on
partitions). Projections then need no weight transposes:
  K^T = Wk^T x^T   (lhsT = Wk chunk, rhs = x^T chunk)
  V   = x Wv       (lhsT = x^T chunk, rhs = Wv chunk)
Scores are computed transposed ([k, q], k on partitions) so softmax's
denominator comes from a ones-column appended to V (row 64 of the attention
output accumulator), and A^T is directly consumable by the A@V matmul.
exp() runs on the scalar engine with the 1/sqrt(dk) folded into its scale.
The normalized per-head outputs O^T are exactly the lhsT the output
projection wants, so no transposes are needed anywhere except on the input x.

Matmul operands are stored as fp16 (10-bit mantissa; measured end-to-end
absmax relative error ~4e-4): unlike f32r this uses the true MAC path, so
the PE clock-gate (HAM) warms to 2.4 GHz and fast weight load applies.
All accumulation is fp32 in PSUM; softmax denominators/reciprocals are fp32.

PSUM budget (8 banks): one shared pool of [128,512] tiles (bufs=6) serves
transposes, projections, scores, the denominator broadcast and the output
projection; a 2-buffer pool holds the per-head attention accumulators.
"""

from contextlib import ExitStack

import numpy as np

import concourse.bass as bass
import concourse.tile as tile
from concourse import bacc, mybir
from concourse.bass_utils import run_bass_kernel_spmd

N_CORES = 8
B, S, D, H, DK = 2, 4096, 512, 8, 64
QL = B * S // N_CORES          # 1024 query rows per core
P = 128
NT_S = S // P                  # 32 sequence tiles
NT_D = D // P                  # 4 d-model chunks
QC = QL // 512                 # 2 query chunks of 512
F32 = mybir.dt.float32
F32R = mybir.dt.float32r
F16 = mybir.dt.float16
EXP = mybir.ActivationFunctionType.Exp

# "f16" (10 mantissa bits, 2.4 GHz MAC path + FWL), "f32r" (13 bits but
# pinned at the 1.2 GHz throttled clock), "f32" (exact, 4 cycles/row).
MM_DTYPE = "f16"
DTM = {"f32r": F32R, "f16": F16, "f32": F32}[MM_DTYPE]


def _emit(ctx: ExitStack, tc: tile.TileContext, io: dict):
    nc = tc.nc
    xb, xq = io["xb"], io["xq"]
    wq, wk, wv, wo = io["wq"], io["wk"], io["wv"], io["wo"]
    bq, bk, bv, bo = io["bq"], io["bk"], io["bv"], io["bo"]
    ident = io["ident"]
    out = io["out"]

    mm = nc.tensor.matmul

    # ---- pools persistent across the whole kernel ------------------------
    consts = ctx.enter_context(tc.tile_pool(name="consts", bufs=1))
    xt_pool = ctx.enter_context(tc.tile_pool(name="xt", bufs=1))
    qt_pool = ctx.enter_context(tc.tile_pool(name="qt", bufs=4))
    # PSUM: shared [128,1024] pool (3 bufs x 2 banks) + attention
    # accumulators (2 banks). Projections use [0:512] slices of the pool.
    ps_pool = ctx.enter_context(tc.tile_pool(name="ps", bufs=3, space="PSUM"))
    o_pool = ctx.enter_context(tc.tile_pool(name="o", bufs=2, space="PSUM"))

    def psum1024(dt=F32):
        return ps_pool.tile([P, 1024], dt, tag="ps", name="ps")

    def psum512(dt=F32):
        return psum1024(dt)[:, 0:512]

    # ---- constants --------------------------------------------------------
    ident_f32 = consts.tile([P, P], F32, tag="ident32")
    nc.sync.dma_start(out=ident_f32[:], in_=ident[:])
    ident_sb = consts.tile([P, P], DTM, tag="ident")
    nc.vector.tensor_copy(out=ident_sb[:], in_=ident_f32[:])
    ones_f32 = consts.tile([P, 1], F32, tag="ones_f32")
    nc.vector.memset(ones_f32[:], 1.0)
    ones_sb = consts.tile([1, 512], DTM, tag="ones")
    nc.vector.tensor_copy(out=ones_sb[:], in_=ones_f32[0:1, 0:1].broadcast_to([1, 512]))
    # a f32 ones row living on partition 64 (denominator broadcast lhsT)
    ones64_sb = consts.tile([65, 64], F32, tag="ones64")
    nc.vector.memset(ones64_sb[64:65, :], 1.0)
    bias_sb = {}
    with tc.tile_pool(name="stg0", bufs=2) as stg0:
        for nm, ap in (("bq", bq), ("bk", bk), ("bv", bv), ("bo", bo)):
            st = stg0.tile([1, D], F32, tag="bstg")
            nc.sync.dma_start(out=st[:], in_=ap[:])
            t = consts.tile([1, D], DTM, tag=nm)
            nc.vector.tensor_copy(out=t[:], in_=st[:])
            bias_sb[nm] = t

    xT = xt_pool.tile([P, NT_D * S], DTM, tag="xT")
    wq_r = wq.rearrange("(dc p) m -> p dc m", p=P)
    wk_r = wk.rearrange("(dc p) m -> p dc m", p=P)
    wv_r = wv.rearrange("(dc p) m -> p dc m", p=P)
    QT = []

    with tc.tile_pool(name="xq", bufs=1) as xq_pool:
        xqT = xq_pool.tile([P, NT_D * QL], DTM, tag="xqT")

        # ---- stage A: x^T and xq^T via PE transposes ---------------------
        with tc.tile_pool(name="xn", bufs=3) as xn_pool:
            for src_ap, ntile, dst in ((xb, NT_S, xT), (xq, QL // P, xqT)):
                for st in range(ntile):
                    xn = xn_pool.tile([P, D], F32, tag="xn")
                    nc.sync.dma_start(out=xn[:], in_=src_ap[st * P:(st + 1) * P, :])
                    xh = xn_pool.tile([P, D], DTM, tag="xh")
                    nc.vector.tensor_copy(out=xh[:], in_=xn[:])
                    tp = psum512(DTM)
                    for dc in range(NT_D):
                        nc.tensor.transpose(
                            tp[:, dc * P:(dc + 1) * P],
                            xh[:, dc * P:(dc + 1) * P],
                            ident_sb[:],
                        )
                    dst_ap = dst[:, :].rearrange("p (dc s) -> p dc s", dc=NT_D)
                    nc.vector.tensor_copy(
                        out=dst_ap[:, :, st * P:(st + 1) * P],
                        in_=tp[:, :].rearrange("p (dc j) -> p dc j", dc=NT_D),
                    )

        # ---- stage B: Q^T for all 4 head pairs ---------------------------
        with (
            tc.tile_pool(name="wqp", bufs=1) as wq_pool,
            tc.tile_pool(name="stgb", bufs=2) as stgb,
        ):
            for pr in range(4):
                wst = stgb.tile([P, NT_D * P], F32, tag="wstg")
                nc.sync.dma_start(
                    out=wst[:, :].rearrange("p (dc m) -> p dc m", dc=NT_D),
                    in_=wq_r[:, :, pr * P:(pr + 1) * P],
                )
                wqp = wq_pool.tile([P, NT_D * P], DTM, tag="wq")
                nc.vector.tensor_copy(out=wqp[:], in_=wst[:])
                qt = qt_pool.tile([P, QL], DTM, tag="QT")
                for qc in range(QC):
                    ps = psum512()
                    for dc in range(NT_D):
                        mm(ps[:], wqp[:, dc * P:(dc + 1) * P],
                           xqT[:, dc * QL + qc * 512:dc * QL + (qc + 1) * 512],
                           start=(dc == 0), stop=False)
                    mm(ps[:], bias_sb["bq"][0:1, pr * P:(pr + 1) * P],
                       ones_sb[0:1, :], start=False, stop=True)
                    nc.vector.tensor_copy(out=qt[:, qc * 512:(qc + 1) * 512],
                                          in_=ps[:])
                QT.append(qt)

    # ---- stage C: per 4-head group: V, then per pair K^T + attention -----
    OT = []  # per-head [64, QL] normalized attention outputs (transposed)
    with tc.tile_pool(name="ot", bufs=8) as ot_pool:
        with (
            tc.tile_pool(name="wkv", bufs=1) as wkv_pool,
            tc.tile_pool(name="stgc", bufs=1) as stgc,
            tc.tile_pool(name="kt", bufs=1) as kt_pool,
            tc.tile_pool(name="v", bufs=1) as v_pool,
            tc.tile_pool(name="e", bufs=6) as e_pool,
            tc.tile_pool(name="rc", bufs=4) as rc_pool,
        ):
            for g in range(2):
                # V for the group's 4 heads, augmented with a ones column:
                # vaug[:, st*260 + hl*65 + (0..63)] = V[st block, head hl]
                # vaug[:, st*260 + hl*65 + 64]      = 1.0
                wst = stgc.tile([P, NT_D * 256], F32, tag="wstg")
                nc.sync.dma_start(
                    out=wst[:, :].rearrange("p (dc m) -> p dc m", dc=NT_D),
                    in_=wv_r[:, :, g * 256:(g + 1) * 256],
                )
                wvg = wkv_pool.tile([P, NT_D * 256], DTM, tag="wv")
                nc.vector.tensor_copy(out=wvg[:], in_=wst[:])
                vaug = v_pool.tile([P, NT_S * 260], DTM, tag="vaug")
                nc.vector.tensor_copy(
                    out=vaug[:, :].rearrange("p (t h e) -> p t h e",
                                             t=NT_S, h=4)[:, :, :, 64:65],
                    in_=ones_f32[:, 0:1].broadcast_to([P, NT_S, 4, 1]),
                )
                for st in range(NT_S):
                    ps = psum512()
                    for dc in range(NT_D):
                        mm(ps[:, 0:256],
                           xT[:, dc * S + st * P:dc * S + (st + 1) * P],
                           wvg[:, dc * 256:(dc + 1) * 256],
                           start=(dc == 0), stop=False)
                    mm(ps[:, 0:256], ones_sb[0:1, 0:P],
                       bias_sb["bv"][0:1, g * 256:(g + 1) * 256],
                       start=False, stop=True)
                    dst = vaug[:, st * 260:(st + 1) * 260]
                    dst = dst.rearrange("p (h e) -> p h e", h=4)[:, :, 0:64]
                    nc.vector.tensor_copy(
                        out=dst,
                        in_=ps[:, 0:256].rearrange("p (h e) -> p h e", h=4),
                    )

                for pi in range(2):
                    pr = 2 * g + pi
                    wst = stgc.tile([P, NT_D * 256], F32, tag="wstg")
                    nc.sync.dma_start(
                        out=wst[:, 0:NT_D * P].rearrange("p (dc m) -> p dc m",
                                                         dc=NT_D),
                        in_=wk_r[:, :, pr * P:(pr + 1) * P],
                    )
                    wkp = wkv_pool.tile([P, NT_D * P], DTM, tag="wk")
                    nc.vector.tensor_copy(out=wkp[:], in_=wst[:, 0:NT_D * P])
                    kt = kt_pool.tile([P, S], DTM, tag="KT")
                    for sc in range(8):
                        ps = psum512()
                        for dc in range(NT_D):
                            mm(ps[:], wkp[:, dc * P:(dc + 1) * P],
                               xT[:, dc * S + sc * 512:dc * S + (sc + 1) * 512],
                               start=(dc == 0), stop=False)
                        mm(ps[:], bias_sb["bk"][0:1, pr * P:(pr + 1) * P],
                           ones_sb[0:1, :], start=False, stop=True)
                        nc.vector.tensor_copy(out=kt[:, sc * 512:(sc + 1) * 512],
                                              in_=ps[:])

                    ot0 = ot_pool.tile([64, QL], DTM, tag="OT")
                    ot1 = ot_pool.tile([64, QL], DTM, tag="OT")
                    OT += [ot0, ot1]
                    qt = QT[pr]
                    hl0, hl1 = 2 * pi, 2 * pi + 1
                    for qc in range(QC):
                        qsl = slice(qc * 512, (qc + 1) * 512)
                        o0 = o_pool.tile([65, 512], F32, tag="O")
                        o1 = o_pool.tile([65, 512], F32, tag="O")

                        def emit_av(sk, ea0, ea1):
                            for j in range(2):
                                ktile = sk * 2 + j
                                st_ = ktile * 260
                                esl = slice(j * 512, (j + 1) * 512)
                                fl = dict(start=(ktile == 0),
                                          stop=(ktile == NT_S - 1))
                                mm(o0[:], vaug[:, st_ + hl0 * 65:
                                               st_ + hl0 * 65 + 65],
                                   ea0[:, esl], **fl)
                                mm(o1[:], vaug[:, st_ + hl1 * 65:
                                               st_ + hl1 * 65 + 65],
                                   ea1[:, esl], **fl)

                        # Software-pipelined emission: the previous
                        # super-k's A@V matmuls get lower scheduling
                        # priority than the current score pair, so the PE
                        # stream is [..AV(sk-1).., sc_h0, sc_h1, ..] and
                        # the paired heads (rows 0-63 / 64-127) stream
                        # through the array concurrently.
                        prev = None
                        for sk in range(NT_S // 2):
                            if prev is not None:
                                emit_av(sk - 1, *prev)
                            sp0 = psum1024()
                            sp1 = psum1024()
                            for j in range(2):
                                ktile = sk * 2 + j
                                ksl = slice(ktile * P, (ktile + 1) * P)
                                jsl = slice(j * 512, (j + 1) * 512)
                                mm(sp0[:, jsl], kt[0:64, ksl], qt[0:64, qsl])
                                mm(sp1[:, jsl], kt[64:128, ksl],
                                   qt[64:128, qsl])
                            ea0 = e_pool.tile([P, 1024], DTM, tag="ea")
                            ea1 = e_pool.tile([P, 1024], DTM, tag="ea")
                            nc.scalar.activation(ea0[:], sp0[:], EXP, scale=0.125)
                            nc.scalar.activation(ea1[:], sp1[:], EXP, scale=0.125)
                            prev = (ea0, ea1)
                        emit_av(NT_S // 2 - 1, *prev)
                        # normalize: O[0:64] * (1 / O[64]) broadcast down.
                        # Copy O out of PSUM immediately (frees the bank),
                        # then run the denominator chain out of SBUF.
                        for o_ps, ot in ((o0, ot0), (o1, ot1)):
                            osb = rc_pool.tile([65, 512], F32, tag="osb")
                            nc.vector.tensor_copy(out=osb[:], in_=o_ps[:])
                            bc = psum512()
                            mm(bc[0:64, :], ones64_sb[64:65, :], osb[64:65, :])
                            rbc = rc_pool.tile([64, 512], F32, tag="rbc")
                            nc.vector.reciprocal(out=rbc[:], in_=bc[0:64, :])
                            nc.vector.tensor_mul(ot[:, qsl], osb[0:64, :], rbc[:])

        # ---- stage D: output projection Y = concat_h(O_h) @ Wo + bo ------
        with (
            tc.tile_pool(name="wo", bufs=8) as wo_pool,
            tc.tile_pool(name="y", bufs=2) as y_pool,
        ):
            wo_sb = []
            for h in range(H):
                wst = y_pool.tile([64, D], F32, tag="wostg")
                nc.sync.dma_start(out=wst[:], in_=wo[h * 64:(h + 1) * 64, :])
                woh = wo_pool.tile([64, D], DTM, tag="wo")
                nc.vector.tensor_copy(out=woh[:], in_=wst[:])
                wo_sb.append(woh)
            for qt_i in range(QL // P):
                ps = psum512()
                for h in range(H):
                    mm(ps[:], OT[h][:, qt_i * P:(qt_i + 1) * P], wo_sb[h][:],
                       start=(h == 0), stop=False)
                mm(ps[:], ones_sb[0:1, 0:P], bias_sb["bo"][0:1, :],
                   start=False, stop=True)
                ysb = y_pool.tile([P, D], F32, tag="y")
                nc.vector.tensor_copy(out=ysb[:], in_=ps[:])
                nc.sync.dma_start(out=out[qt_i * P:(qt_i + 1) * P, :], in_=ysb[:])


def build():
    nc = bacc.Bacc("TRN2", target_bir_lowering=False, debug=False,
                   num_devices=N_CORES)
    io = {}
    for nm, shape in (("xb", [S, D]), ("xq", [QL, D]), ("wq", [D, D]),
                      ("wk", [D, D]), ("wv", [D, D]), ("wo", [D, D]),
                      ("bq", [1, D]), ("bk", [1, D]), ("bv", [1, D]),
                      ("bo", [1, D]), ("ident", [P, P])):
        io[nm] = nc.dram_tensor(nm, shape, F32, kind="ExternalInput").ap()
    io["out"] = nc.dram_tensor("out", [QL, D], F32, kind="ExternalOutput").ap()
    with tile.TileContext(nc) as tc:
        with ExitStack() as ctx:
            _emit(ctx, tc, io)
    nc.compile()
    return nc


def make_in_maps(inputs):
    f = lambda a: np.ascontiguousarray(np.asarray(a, dtype=np.float32))
    x = f(inputs["x"])
    fixed = {
        "wq": f(inputs["Wq"]), "wk": f(inputs["Wk"]), "wv": f(inputs["Wv"]),
        "wo": f(inputs["Wo"]),
        "bq": f(inputs["bq"]).reshape(1, D), "bk": f(inputs["bk"]).reshape(1, D),
        "bv": f(inputs["bv"]).reshape(1, D), "bo": f(inputs["bo"]).reshape(1, D),
        "ident": np.eye(P, dtype=np.float32),
    }
    in_maps = []
    for c in range(N_CORES):
        b, qs = c // 4, (c % 4) * QL
        in_maps.append({"xb": x[b], "xq": x[b, qs:qs + QL], **fixed})
    return in_maps


_CACHE = {}
LAST_EXEC_NS = None


def run(inputs, trace=False):
    global LAST_EXEC_NS
    if "nc" not in _CACHE:
        _CACHE["nc"] = build()
    nc = _CACHE["nc"]
    kw = {}
    if trace:
        import sys, types
        if "antenv.axon_hooks" not in sys.modules:
            sys.path.insert(0, "/root/.axon_site")
            try:
                from trn_agent_boot.trn_boot import _ntff_profile_via_ctypes
                hook = _ntff_profile_via_ctypes("/opt/axon/libaxon_pjrt.so")
                mod = types.ModuleType("antenv.axon_hooks")
                mod.get_axon_ntff_profile_hook = lambda: hook
                mod.set_axon_ntff_profile_hook = lambda h: None
                sys.modules["antenv.axon_hooks"] = mod
            except Exception:
                pass
        kw = dict(trace=True, trace_cores=[0])
    res = run_bass_kernel_spmd(nc, make_in_maps(inputs),
                               core_ids=list(range(N_CORES)), **kw)
    if trace:
        LAST_EXEC_NS = res.exec_time_ns
    out = np.empty((B, S, D), np.float32)
    for c in range(N_CORES):
        b, qs = c // 4, (c % 4) * QL
        out[b, qs:qs + QL] = res.results[c]["out"]
    return out


def kernel(**inputs) -> np.ndarray:
    return run(inputs, trace=False)


# revision 15
# speedup vs baseline: 1.3887x; 1.0897x over previous
"""Multi-head self-attention Trainium2 Bass kernel (8-core SPMD).

Sharding: data-parallel over query rows. The flattened (B*S, D) = (8192, 512)
query space is split into 8 blocks of 1024 rows; core c handles batch c//4,
query rows (c%4)*1024 .. +1024. Each core recomputes K/V for its whole batch
(4-way duplicated) which avoids any cross-core communication; host-side
gather is a pure concatenation.

Layout strategy: activations live transposed in SBUF ([D, S], d on
partitions). Projections then need no weight transposes:
  K^T = Wk^T x^T   (lhsT = Wk chunk, rhs = x^T chunk)
  V   = x Wv       (lhsT = x^T chunk, rhs = Wv chunk)
Scores are computed transposed ([k, q], k on partitions) so softmax's
denominator comes from a ones-column appended to V (row 64 of the attention
output accumulator), and A^T is directly consumable by the A@V matmul.
exp() runs on the scalar engine with the 1/sqrt(dk) folded into its scale.
The normalized per-head outputs O^T are exactly the lhsT the output
projection wants, so no transposes are needed anywhere except on the input x.

Matmul operands are stored as fp16 (10-bit mantissa; measured end-to-end
absmax relative error ~4e-4): this is the true MAC path, so the PE
clock-gate can warm to 2.4 GHz and fast weight load applies. All
accumulation is fp32 in PSUM; softmax denominators/reciprocals are fp32.

The two heads of a pair are kept adjacent in the PE stream via scheduler
dependency edges; their score matmuls occupy PE row strips 0-63 / 64-127
and execute concurrently in the array.
"""

from contextlib import ExitStack

import numpy as np

import concourse.bass as bass
import concourse.tile as tile
from concourse import bacc, mybir
from concourse.bass import _add_dep_helper
from concourse.bass_utils import run_bass_kernel_spmd

N_CORES = 8
B, S, D, H, DK = 2, 4096, 512, 8, 64
QL = B * S // N_CORES          # 1024 query rows per core
P = 128
NT_S = S // P                  # 32 sequence tiles
NT_D = D // P                  # 4 d-model chunks
QC = QL // 512                 # 2 query chunks of 512
VW = H * 65                    # 520: per-k-tile width of the augmented V
F32 = mybir.dt.float32
F32R = mybir.dt.float32r
F16 = mybir.dt.float16
EXP = mybir.ActivationFunctionType.Exp

# "f16" (10 mantissa bits, 2.4 GHz MAC path + FWL), "f32r" (13 bits but
# pinned at the 1.2 GHz throttled clock), "f32" (exact, 4 cycles/row).
MM_DTYPE = "f16"
DTM = {"f32r": F32R, "f16": F16, "f32": F32}[MM_DTYPE]


def _emit(ctx: ExitStack, tc: tile.TileContext, io: dict):
    nc = tc.nc
    xb, xq = io["xb"], io["xq"]
    wq, wk, wv, wo = io["wq"], io["wk"], io["wv"], io["wo"]
    bq, bk, bv, bo = io["bq"], io["bk"], io["bv"], io["bo"]
    ident = io["ident"]
    out = io["out"]

    mm = nc.tensor.matmul

    # ---- pools persistent across the whole kernel ------------------------
    consts = ctx.enter_context(tc.tile_pool(name="consts", bufs=1))
    xt_pool = ctx.enter_context(tc.tile_pool(name="xt", bufs=1))
    qt_pool = ctx.enter_context(tc.tile_pool(name="qt", bufs=4))
    v_pool = ctx.enter_context(tc.tile_pool(name="v", bufs=1))
    # PSUM: shared [128,1024] pool (3 bufs x 2 banks) + attention
    # accumulators (2 banks). Projections use [0:512] slices of the pool.
    ps_pool = ctx.enter_context(tc.tile_pool(name="ps", bufs=3, space="PSUM"))
    o_pool = ctx.enter_context(tc.tile_pool(name="o", bufs=2, space="PSUM"))

    def psum1024(dt=F32):
        return ps_pool.tile([P, 1024], dt, tag="ps", name="ps")

    def psum512(dt=F32):
        return psum1024(dt)[:, 0:512]

    # ---- constants --------------------------------------------------------
    ident_f32 = consts.tile([P, P], F32, tag="ident32")
    nc.sync.dma_start(out=ident_f32[:], in_=ident[:])
    ident_sb = consts.tile([P, P], DTM, tag="ident")
    nc.vector.tensor_copy(out=ident_sb[:], in_=ident_f32[:])
    ones_f32 = consts.tile([P, 1], F32, tag="ones_f32")
    nc.vector.memset(ones_f32[:], 1.0)
    ones_sb = consts.tile([1, 512], DTM, tag="ones")
    nc.vector.tensor_copy(out=ones_sb[:], in_=ones_f32[0:1, 0:1].broadcast_to([1, 512]))
    # a f32 ones row living on partition 64 (denominator broadcast lhsT)
    ones64_sb = consts.tile([65, 64], F32, tag="ones64")
    nc.vector.memset(ones64_sb[64:65, :], 1.0)
    # biases: bv/bo as [1, D] rows (free-dim biases, applied via rank-1
    # matmuls); bk/bq as [128, 4] per-partition columns (fused into the
    # PSUM->SBUF copies on the vector engine).
    bias_sb = {}
    with tc.tile_pool(name="stg0", bufs=2) as stg0:
        for nm, ap in (("bv", bv), ("bo", bo)):
            st = stg0.tile([1, D], F32, tag="bstg")
            nc.sync.dma_start(out=st[:], in_=ap[:])
            t = consts.tile([1, D], DTM, tag=nm)
            nc.vector.tensor_copy(out=t[:], in_=st[:])
            bias_sb[nm] = t
    bkT = consts.tile([P, 4], F32, tag="bkT")
    nc.sync.dma_start(out=bkT[:], in_=bk.rearrange("o (pr p) -> p (o pr)", p=P))
    bqT = consts.tile([P, 4], F32, tag="bqT")
    nc.sync.dma_start(out=bqT[:], in_=bq.rearrange("o (pr p) -> p (o pr)", p=P))

    xT = xt_pool.tile([P, NT_D * S], DTM, tag="xT")
    wq_r = wq.rearrange("(dc p) m -> p dc m", p=P)
    wk_r = wk.rearrange("(dc p) m -> p dc m", p=P)
    wv_r = wv.rearrange("(dc p) m -> p dc m", p=P)
    QT = []

    with tc.tile_pool(name="xq", bufs=1) as xq_pool:
        xqT = xq_pool.tile([P, NT_D * QL], DTM, tag="xqT")

        # ---- stage A: x^T and xq^T via PE transposes ---------------------
        with tc.tile_pool(name="xn", bufs=3) as xn_pool:
            for src_ap, ntile, dst in ((xb, NT_S, xT), (xq, QL // P, xqT)):
                for st in range(ntile):
                    xn = xn_pool.tile([P, D], F32, tag="xn")
                    nc.sync.dma_start(out=xn[:], in_=src_ap[st * P:(st + 1) * P, :])
                    xh = xn_pool.tile([P, D], DTM, tag="xh")
                    nc.vector.tensor_copy(out=xh[:], in_=xn[:])
                    tp = psum512(DTM)
                    for dc in range(NT_D):
                        nc.tensor.transpose(
                            tp[:, dc * P:(dc + 1) * P],
                            xh[:, dc * P:(dc + 1) * P],
                            ident_sb[:],
                        )
                    dst_ap = dst[:, :].rearrange("p (dc s) -> p dc s", dc=NT_D)
                    nc.vector.tensor_copy(
                        out=dst_ap[:, :, st * P:(st + 1) * P],
                        in_=tp[:, :].rearrange("p (dc j) -> p dc j", dc=NT_D),
                    )

        # ---- stage B: Q^T for all 4 head pairs ---------------------------
        with (
            tc.tile_pool(name="wqp", bufs=1) as wq_pool,
            tc.tile_pool(name="stgb", bufs=2) as stgb,
        ):
            for pr in range(4):
                wst = stgb.tile([P, NT_D * P], F32, tag="wstg")
                nc.sync.dma_start(
                    out=wst[:, :].rearrange("p (dc m) -> p dc m", dc=NT_D),
                    in_=wq_r[:, :, pr * P:(pr + 1) * P],
                )
                wqp = wq_pool.tile([P, NT_D * P], DTM, tag="wq")
                nc.vector.tensor_copy(out=wqp[:], in_=wst[:])
                qt = qt_pool.tile([P, QL], DTM, tag="QT")
                for qc in range(QC):
                    ps = psum512()
                    for dc in range(NT_D):
                        mm(ps[:], wqp[:, dc * P:(dc + 1) * P],
                           xqT[:, dc * QL + qc * 512:dc * QL + (qc + 1) * 512],
                           start=(dc == 0), stop=(dc == NT_D - 1))
                    nc.vector.tensor_scalar_add(
                        out=qt[:, qc * 512:(qc + 1) * 512], in0=ps[:],
                        scalar1=bqT[:, pr:pr + 1],
                    )
                QT.append(qt)

    # ---- stage C: V for all heads, then per pair K^T + attention ---------
    OT = []  # per-head [64, QL] normalized attention outputs (transposed)
    with tc.tile_pool(name="ot", bufs=8) as ot_pool:
        with (
            tc.tile_pool(name="wkv", bufs=2) as wkv_pool,
            tc.tile_pool(name="stgc", bufs=2) as stgc,
            tc.tile_pool(name="kt", bufs=1) as kt_pool,
            tc.tile_pool(name="e", bufs=6) as e_pool,
            tc.tile_pool(name="rc", bufs=4) as rc_pool,
        ):
            # V for all 8 heads, augmented with a ones column per head:
            # vaug[:, kt*520 + h*65 + (0..63)] = V[k-tile kt, head h]
            # vaug[:, kt*520 + h*65 + 64]      = 1.0
            wst = stgc.tile([P, NT_D * 512], F32, tag="wvstg")
            nc.sync.dma_start(
                out=wst[:, :].rearrange("p (dc m) -> p dc m", dc=NT_D),
                in_=wv_r[:, :, :],
            )
            wvg = wkv_pool.tile([P, NT_D * 512], DTM, tag="wv")
            nc.vector.tensor_copy(out=wvg[:], in_=wst[:])
            vaug = v_pool.tile([P, NT_S * VW], DTM, tag="vaug")
            nc.vector.tensor_copy(
                out=vaug[:, :].rearrange("p (t h e) -> p t h e",
                                         t=NT_S, h=H)[:, :, :, 64:65],
                in_=ones_f32[:, 0:1].broadcast_to([P, NT_S, H, 1]),
            )
            for st in range(NT_S):
                ps = psum512()
                for dc in range(NT_D):
                    mm(ps[:], xT[:, dc * S + st * P:dc * S + (st + 1) * P],
                       wvg[:, dc * 512:(dc + 1) * 512],
                       start=(dc == 0), stop=False)
                mm(ps[:], ones_sb[0:1, 0:P], bias_sb["bv"][0:1, :],
                   start=False, stop=True)
                dst = vaug[:, st * VW:(st + 1) * VW]
                dst = dst.rearrange("p (h e) -> p h e", h=H)[:, :, 0:64]
                nc.vector.tensor_copy(
                    out=dst, in_=ps[:].rearrange("p (h e) -> p h e", h=H)
                )

            for pr in range(4):
                hl0, hl1 = 2 * pr, 2 * pr + 1
                wst = stgc.tile([P, NT_D * 512], F32, tag="wvstg")
                nc.sync.dma_start(
                    out=wst[:, 0:NT_D * P].rearrange("p (dc m) -> p dc m",
                                                     dc=NT_D),
                    in_=wk_r[:, :, pr * P:(pr + 1) * P],
                )
                wkp = wkv_pool.tile([P, NT_D * P], DTM, tag="wk")
                nc.vector.tensor_copy(out=wkp[:], in_=wst[:, 0:NT_D * P])
                kt = kt_pool.tile([P, S], DTM, tag="KT")
                for sc in range(8):
                    ps = psum512()
                    for dc in range(NT_D):
                        mm(ps[:], wkp[:, dc * P:(dc + 1) * P],
                           xT[:, dc * S + sc * 512:dc * S + (sc + 1) * 512],
                           start=(dc == 0), stop=(dc == NT_D - 1))
                    nc.vector.tensor_scalar_add(
                        out=kt[:, sc * 512:(sc + 1) * 512], in0=ps[:],
                        scalar1=bkT[:, pr:pr + 1],
                    )

                ot0 = ot_pool.tile([64, QL], DTM, tag="OT")
                ot1 = ot_pool.tile([64, QL], DTM, tag="OT")
                OT += [ot0, ot1]
                qt = QT[pr]
                for qc in range(QC):
                    qsl = slice(qc * 512, (qc + 1) * 512)
                    o0 = o_pool.tile([65, 512], F32, tag="O")
                    o1 = o_pool.tile([65, 512], F32, tag="O")

                    def emit_av(sk, ea0, ea1, gate):
                        for j in range(2):
                            ktile = sk * 2 + j
                            st_ = ktile * VW
                            esl = slice(j * 512, (j + 1) * 512)
                            fl = dict(start=(ktile == 0),
                                      stop=(ktile == NT_S - 1))
                            i0 = mm(o0[:], vaug[:, st_ + hl0 * 65:
                                                st_ + hl0 * 65 + 65],
                                    ea0[:, esl], **fl)
                            i1 = mm(o1[:], vaug[:, st_ + hl1 * 65:
                                                st_ + hl1 * 65 + 65],
                                    ea1[:, esl], **fl)
                            if gate is not None:
                                # order the A@V matmuls after the next score
                                # pair so the paired heads stay adjacent in
                                # the PE stream (concurrent row strips).
                                _add_dep_helper(i0.ins, gate.ins, sync=False,
                                                reason="attn pipeline order")
                                _add_dep_helper(i1.ins, gate.ins, sync=False,
                                                reason="attn pipeline order")

                    prev = None
                    for sk in range(NT_S // 2):
                        sp0 = psum1024()
                        sp1 = psum1024()
                        gate = None
                        for j in range(2):
                            ktile = sk * 2 + j
                            ksl = slice(ktile * P, (ktile + 1) * P)
                            jsl = slice(j * 512, (j + 1) * 512)
                            mm(sp0[:, jsl], kt[0:64, ksl], qt[0:64, qsl])
                            g = mm(sp1[:, jsl], kt[64:128, ksl],
                                   qt[64:128, qsl])
                            if j == 0:
                                gate = g
                        if prev is not None:
                            emit_av(sk - 1, *prev, gate)
                        ea0 = e_pool.tile([P, 1024], DTM, tag="ea")
                        ea1 = e_pool.tile([P, 1024], DTM, tag="ea")
                        nc.scalar.activation(ea0[:], sp0[:], EXP, scale=0.125)
                        nc.scalar.activation(ea1[:], sp1[:], EXP, scale=0.125)
                        prev = (ea0, ea1)
                    emit_av(NT_S // 2 - 1, *prev, None)
                    # normalize: O[0:64] * (1 / O[64]) broadcast down.
                    # Copy O out of PSUM immediately (frees the bank),
                    # then run the denominator chain out of SBUF.
                    for o_ps, ot in ((o0, ot0), (o1, ot1)):
                        osb = rc_pool.tile([65, 512], F32, tag="osb")
                        nc.vector.tensor_copy(out=osb[:], in_=o_ps[:])
                        bc = psum512()
                        mm(bc[0:64, :], ones64_sb[64:65, :], osb[64:65, :])
                        rbc = rc_pool.tile([64, 512], F32, tag="rbc")
                        nc.vector.reciprocal(out=rbc[:], in_=bc[0:64, :])
                        nc.vector.tensor_mul(ot[:, qsl], osb[0:64, :], rbc[:])

        # ---- stage D: output projection Y = concat_h(O_h) @ Wo + bo ------
        with (
            tc.tile_pool(name="wo", bufs=8) as wo_pool,
            tc.tile_pool(name="y", bufs=2) as y_pool,
        ):
            wo_sb = []
            for h in range(H):
                wst = y_pool.tile([64, D], F32, tag="wostg")
                nc.sync.dma_start(out=wst[:], in_=wo[h * 64:(h + 1) * 64, :])
                woh = wo_pool.tile([64, D], DTM, tag="wo")
                nc.vector.tensor_copy(out=woh[:], in_=wst[:])
                wo_sb.append(woh)
            for qt_i in range(QL // P):
                ps = psum512()
                for h in range(H):
                    mm(ps[:], OT[h][:, qt_i * P:(qt_i + 1) * P], wo_sb[h][:],
                       start=(h == 0), stop=False)
                mm(ps[:], ones_sb[0:1, 0:P], bias_sb["bo"][0:1, :],
                   start=False, stop=True)
                ysb = y_pool.tile([P, D], F32, tag="y")
                nc.vector.tensor_copy(out=ysb[:], in_=ps[:])
                nc.sync.dma_start(out=out[qt_i * P:(qt_i + 1) * P, :], in_=ysb[:])


def build():
    nc = bacc.Bacc("TRN2", target_bir_lowering=False, debug=False,
                   num_devices=N_CORES)
    io = {}
    for nm, shape in (("xb", [S, D]), ("xq", [QL, D]), ("wq", [D, D]),
                      ("wk", [D, D]), ("wv", [D, D]), ("wo", [D, D]),
                      ("bq", [1, D]), ("bk", [1, D]), ("bv", [1, D]),
                      ("bo", [1, D]), ("ident", [P, P])):
        io[nm] = nc.dram_tensor(nm, shape, F32, kind="ExternalInput").ap()
    io["out"] = nc.dram_tensor("out", [QL, D], F32, kind="ExternalOutput").ap()
    with tile.TileContext(nc) as tc:
        with ExitStack() as ctx:
            _emit(ctx, tc, io)
    nc.compile()
    return nc


def make_in_maps(inputs):
    f = lambda a: np.ascontiguousarray(np.asarray(a, dtype=np.float32))
    x = f(inputs["x"])
    fixed = {
        "wq": f(inputs["Wq"]), "wk": f(inputs["Wk"]), "wv": f(inputs["Wv"]),
        "wo": f(inputs["Wo"]),
        "bq": f(inputs["bq"]).reshape(1, D), "bk": f(inputs["bk"]).reshape(1, D),
        "bv": f(inputs["bv"]).reshape(1, D), "bo": f(inputs["bo"]).reshape(1, D),
        "ident": np.eye(P, dtype=np.float32),
    }
    in_maps = []
    for c in range(N_CORES):
        b, qs = c // 4, (c % 4) * QL
        in_maps.append({"xb": x[b], "xq": x[b, qs:qs + QL], **fixed})
    return in_maps


_CACHE = {}
LAST_EXEC_NS = None


def run(inputs, trace=False):
    global LAST_EXEC_NS
    if "nc" not in _CACHE:
        _CACHE["nc"] = build()
    nc = _CACHE["nc"]
    kw = {}
    if trace:
        import sys, types
        if "antenv.axon_hooks" not in sys.modules:
            sys.path.insert(0, "/root/.axon_site")
            try:
                from trn_agent_boot.trn_boot import _ntff_profile_via_ctypes
                hook = _ntff_profile_via_ctypes("/opt/axon/libaxon_pjrt.so")
                mod = types.ModuleType("antenv.axon_hooks")
                mod.get_axon_ntff_profile_hook = lambda: hook
                mod.set_axon_ntff_profile_hook = lambda h: None
                sys.modules["antenv.axon_hooks"] = mod
            except Exception:
                pass
        kw = dict(trace=True, trace_cores=[0])
    res = run_bass_kernel_spmd(nc, make_in_maps(inputs),
                               core_ids=list(range(N_CORES)), **kw)
    if trace:
        LAST_EXEC_NS = res.exec_time_ns
    out = np.empty((B, S, D), np.float32)
    for c in range(N_CORES):
        b, qs = c // 4, (c % 4) * QL
        out[b, qs:qs + QL] = res.results[c]["out"]
    return out


def kernel(**inputs) -> np.ndarray:
    return run(inputs, trace=False)


# revision 16
# speedup vs baseline: 1.6379x; 1.1794x over previous
"""Multi-head self-attention Trainium2 Bass kernel (8-core SPMD).

Sharding: data-parallel over query rows. The flattened (B*S, D) = (8192, 512)
query space is split into 8 blocks of 1024 rows; core c handles batch c//4,
query rows (c%4)*1024 .. +1024. Each core recomputes K/V for its whole batch
(4-way duplicated) which avoids any cross-core communication; host-side
gather is a pure concatenation.

Layout strategy: activations live transposed in SBUF ([D, S], d on
partitions). Projections then need no weight transposes:
  K^T = Wk^T x^T   (lhsT = Wk chunk, rhs = x^T chunk)
  V   = x Wv       (lhsT = x^T chunk, rhs = Wv chunk)
Scores are computed transposed ([k, q], k on partitions) so softmax's
denominator comes from a ones-column appended to V (row 64 of the attention
output accumulator), and A^T is directly consumable by the A@V matmul.
exp() runs on the scalar engine with the 1/sqrt(dk) folded into its scale.
The normalized per-head outputs O^T are exactly the lhsT the output
projection wants, so no transposes are needed anywhere except on the input x.

Matmul operands are stored as fp16 (10-bit mantissa; measured end-to-end
absmax relative error ~4e-4): this is the true MAC path, so the PE
clock-gate can warm to 2.4 GHz and fast weight load applies. All
accumulation is fp32 in PSUM; softmax denominators/reciprocals are fp32.

The two heads of a pair are kept adjacent in the PE stream via scheduler
dependency edges; their score matmuls occupy PE row strips 0-63 / 64-127
and execute concurrently in the array.
"""

from contextlib import ExitStack

import numpy as np

import concourse.bass as bass
import concourse.tile as tile
from concourse import bacc, mybir
from concourse.bass import _add_dep_helper
from concourse.bass_utils import run_bass_kernel_spmd

N_CORES = 8
B, S, D, H, DK = 2, 4096, 512, 8, 64
QL = B * S // N_CORES          # 1024 query rows per core
P = 128
NT_S = S // P                  # 32 sequence tiles
NT_D = D // P                  # 4 d-model chunks
QC = QL // 512                 # 2 query chunks of 512
VW = H * 65                    # 520: per-k-tile width of the augmented V
F32 = mybir.dt.float32
F32R = mybir.dt.float32r
F16 = mybir.dt.float16
EXP = mybir.ActivationFunctionType.Exp

# "f16" (10 mantissa bits, 2.4 GHz MAC path + FWL), "f32r" (13 bits but
# pinned at the 1.2 GHz throttled clock), "f32" (exact, 4 cycles/row).
MM_DTYPE = "f16"
DTM = {"f32r": F32R, "f16": F16, "f32": F32}[MM_DTYPE]


def _emit(ctx: ExitStack, tc: tile.TileContext, io: dict):
    nc = tc.nc
    xb, xq = io["xb"], io["xq"]
    wq, wk, wv, wo = io["wq"], io["wk"], io["wv"], io["wo"]
    bq, bk, bv, bo = io["bq"], io["bk"], io["bv"], io["bo"]
    ident = io["ident"]
    out = io["out"]

    mm = nc.tensor.matmul

    # ---- pools persistent across the whole kernel ------------------------
    consts = ctx.enter_context(tc.tile_pool(name="consts", bufs=1))
    xt_pool = ctx.enter_context(tc.tile_pool(name="xt", bufs=1))
    qt_pool = ctx.enter_context(tc.tile_pool(name="qt", bufs=4))
    v_pool = ctx.enter_context(tc.tile_pool(name="v", bufs=1))
    # PSUM: shared [128,1024] pool (3 bufs x 2 banks) + attention
    # accumulators (2 banks). Projections use [0:512] slices of the pool.
    ps_pool = ctx.enter_context(tc.tile_pool(name="ps", bufs=3, space="PSUM"))
    o_pool = ctx.enter_context(tc.tile_pool(name="o", bufs=2, space="PSUM"))

    def psum1024(dt=F32):
        return ps_pool.tile([P, 1024], dt, tag="ps", name="ps")

    def psum512(dt=F32):
        return psum1024(dt)[:, 0:512]

    # ---- constants --------------------------------------------------------
    ident_f32 = consts.tile([P, P], F32, tag="ident32")
    nc.sync.dma_start(out=ident_f32[:], in_=ident[:])
    ident_sb = consts.tile([P, P], DTM, tag="ident")
    nc.vector.tensor_copy(out=ident_sb[:], in_=ident_f32[:])
    ones_f32 = consts.tile([P, 1], F32, tag="ones_f32")
    nc.vector.memset(ones_f32[:], 1.0)
    ones_sb = consts.tile([1, 512], DTM, tag="ones")
    nc.vector.tensor_copy(out=ones_sb[:], in_=ones_f32[0:1, 0:1].broadcast_to([1, 512]))
    # a f32 ones row living on partition 64 (denominator broadcast lhsT)
    ones64_sb = consts.tile([65, 64], F32, tag="ones64")
    nc.vector.memset(ones64_sb[64:65, :], 1.0)
    # biases: bv/bo as [1, D] rows (free-dim biases, applied via rank-1
    # matmuls); bk/bq as [128, 4] per-partition columns (fused into the
    # PSUM->SBUF copies on the vector engine).
    bias_sb = {}
    with tc.tile_pool(name="stg0", bufs=2) as stg0:
        for nm, ap in (("bv", bv), ("bo", bo)):
            st = stg0.tile([1, D], F32, tag="bstg")
            nc.sync.dma_start(out=st[:], in_=ap[:])
            t = consts.tile([1, D], DTM, tag=nm)
            nc.vector.tensor_copy(out=t[:], in_=st[:])
            bias_sb[nm] = t
    bkT = consts.tile([P, 4], F32, tag="bkT")
    nc.sync.dma_start(out=bkT[:], in_=bk.rearrange("o (pr p) -> p (o pr)", p=P))
    bqT = consts.tile([P, 4], F32, tag="bqT")
    nc.sync.dma_start(out=bqT[:], in_=bq.rearrange("o (pr p) -> p (o pr)", p=P))

    xT = xt_pool.tile([P, NT_D * S], DTM, tag="xT")
    wq_r = wq.rearrange("(dc p) m -> p dc m", p=P)
    wk_r = wk.rearrange("(dc p) m -> p dc m", p=P)
    wv_r = wv.rearrange("(dc p) m -> p dc m", p=P)
    QT = []

    with tc.tile_pool(name="xq", bufs=1) as xq_pool:
        xqT = xq_pool.tile([P, NT_D * QL], DTM, tag="xqT")

        # ---- stage A: x^T and xq^T via PE transposes ---------------------
        with tc.tile_pool(name="xn", bufs=3) as xn_pool:
            for src_ap, ntile, dst in ((xb, NT_S, xT), (xq, QL // P, xqT)):
                for st in range(ntile):
                    xn = xn_pool.tile([P, D], F32, tag="xn")
                    nc.sync.dma_start(out=xn[:], in_=src_ap[st * P:(st + 1) * P, :])
                    xh = xn_pool.tile([P, D], DTM, tag="xh")
                    nc.vector.tensor_copy(out=xh[:], in_=xn[:])
                    tp = psum512(DTM)
                    for dc in range(NT_D):
                        nc.tensor.transpose(
                            tp[:, dc * P:(dc + 1) * P],
                            xh[:, dc * P:(dc + 1) * P],
                            ident_sb[:],
                        )
                    dst_ap = dst[:, :].rearrange("p (dc s) -> p dc s", dc=NT_D)
                    nc.vector.tensor_copy(
                        out=dst_ap[:, :, st * P:(st + 1) * P],
                        in_=tp[:, :].rearrange("p (dc j) -> p dc j", dc=NT_D),
                    )

        # ---- stage B: Q^T for all 4 head pairs ---------------------------
        with (
            tc.tile_pool(name="wqp", bufs=1) as wq_pool,
            tc.tile_pool(name="stgb", bufs=2) as stgb,
        ):
            for pr in range(4):
                wst = stgb.tile([P, NT_D * P], F32, tag="wstg")
                nc.sync.dma_start(
                    out=wst[:, :].rearrange("p (dc m) -> p dc m", dc=NT_D),
                    in_=wq_r[:, :, pr * P:(pr + 1) * P],
                )
                wqp = wq_pool.tile([P, NT_D * P], DTM, tag="wq")
                nc.vector.tensor_copy(out=wqp[:], in_=wst[:])
                qt = qt_pool.tile([P, QL], DTM, tag="QT")
                for qc in range(QC):
                    ps = psum512()
                    for dc in range(NT_D):
                        mm(ps[:], wqp[:, dc * P:(dc + 1) * P],
                           xqT[:, dc * QL + qc * 512:dc * QL + (qc + 1) * 512],
                           start=(dc == 0), stop=(dc == NT_D - 1))
                    nc.vector.tensor_scalar_add(
                        out=qt[:, qc * 512:(qc + 1) * 512], in0=ps[:],
                        scalar1=bqT[:, pr:pr + 1],
                    )
                QT.append(qt)

    # ---- stage C: V for all heads, then per pair K^T + attention ---------
    OT = []  # per-head [64, QL] normalized attention outputs (transposed)
    with tc.tile_pool(name="ot", bufs=8) as ot_pool:
        with (
            tc.tile_pool(name="wkv", bufs=2) as wkv_pool,
            tc.tile_pool(name="stgc", bufs=2) as stgc,
            tc.tile_pool(name="kt", bufs=1) as kt_pool,
            tc.tile_pool(name="e", bufs=6) as e_pool,
            tc.tile_pool(name="rc", bufs=4) as rc_pool,
        ):
            # V for all 8 heads, augmented with a ones column per head:
            # vaug[:, kt*520 + h*65 + (0..63)] = V[k-tile kt, head h]
            # vaug[:, kt*520 + h*65 + 64]      = 1.0
            wst = stgc.tile([P, NT_D * 512], F32, tag="wvstg")
            nc.sync.dma_start(
                out=wst[:, :].rearrange("p (dc m) -> p dc m", dc=NT_D),
                in_=wv_r[:, :, :],
            )
            wvg = wkv_pool.tile([P, NT_D * 512], DTM, tag="wv")
            nc.vector.tensor_copy(out=wvg[:], in_=wst[:])
            vaug = v_pool.tile([P, NT_S * VW], DTM, tag="vaug")
            nc.vector.tensor_copy(
                out=vaug[:, :].rearrange("p (t h e) -> p t h e",
                                         t=NT_S, h=H)[:, :, :, 64:65],
                in_=ones_f32[:, 0:1].broadcast_to([P, NT_S, H, 1]),
            )
            for st in range(NT_S):
                ps = psum512()
                for dc in range(NT_D):
                    mm(ps[:], xT[:, dc * S + st * P:dc * S + (st + 1) * P],
                       wvg[:, dc * 512:(dc + 1) * 512],
                       start=(dc == 0), stop=False)
                mm(ps[:], ones_sb[0:1, 0:P], bias_sb["bv"][0:1, :],
                   start=False, stop=True)
                dst = vaug[:, st * VW:(st + 1) * VW]
                dst = dst.rearrange("p (h e) -> p h e", h=H)[:, :, 0:64]
                nc.vector.tensor_copy(
                    out=dst, in_=ps[:].rearrange("p (h e) -> p h e", h=H)
                )

            for pr in range(4):
                hl0, hl1 = 2 * pr, 2 * pr + 1
                wst = stgc.tile([P, NT_D * 512], F32, tag="wvstg")
                nc.sync.dma_start(
                    out=wst[:, 0:NT_D * P].rearrange("p (dc m) -> p dc m",
                                                     dc=NT_D),
                    in_=wk_r[:, :, pr * P:(pr + 1) * P],
                )
                wkp = wkv_pool.tile([P, NT_D * P], DTM, tag="wk")
                nc.vector.tensor_copy(out=wkp[:], in_=wst[:, 0:NT_D * P])
                kt = kt_pool.tile([P, S], DTM, tag="KT")
                for sc in range(8):
                    ps = psum512()
                    for dc in range(NT_D):
                        mm(ps[:], wkp[:, dc * P:(dc + 1) * P],
                           xT[:, dc * S + sc * 512:dc * S + (sc + 1) * 512],
                           start=(dc == 0), stop=(dc == NT_D - 1))
                    nc.vector.tensor_scalar_add(
                        out=kt[:, sc * 512:(sc + 1) * 512], in0=ps[:],
                        scalar1=bkT[:, pr:pr + 1],
                    )

                ot0 = ot_pool.tile([64, QL], DTM, tag="OT")
                ot1 = ot_pool.tile([64, QL], DTM, tag="OT")
                OT += [ot0, ot1]
                qt = QT[pr]
                for qc in range(QC):
                    qsl = slice(qc * 512, (qc + 1) * 512)
                    o0 = o_pool.tile([65, 512], F32, tag="O")
                    o1 = o_pool.tile([65, 512], F32, tag="O")

                    def emit_av(sk, ea0, ea1, gate):
                        for j in range(2):
                            ktile = sk * 2 + j
                            st_ = ktile * VW
                            esl = slice(j * 512, (j + 1) * 512)
                            fl = dict(start=(ktile == 0),
                                      stop=(ktile == NT_S - 1))
                            i0 = mm(o0[:], vaug[:, st_ + hl0 * 65:
                                                st_ + hl0 * 65 + 65],
                                    ea0[:, esl], **fl)
                            i1 = mm(o1[:], vaug[:, st_ + hl1 * 65:
                                                st_ + hl1 * 65 + 65],
                                    ea1[:, esl], **fl)
                            if gate is not None:
                                # order the A@V matmuls after the next score
                                # pair so the paired heads stay adjacent in
                                # the PE stream (concurrent row strips).
                                _add_dep_helper(i0.ins, gate.ins, sync=False,
                                                reason="attn pipeline order")
                                _add_dep_helper(i1.ins, gate.ins, sync=False,
                                                reason="attn pipeline order")

                    pending = []  # [(sk, ea0, ea1), ...] not yet AV-emitted
                    for sk in range(NT_S // 2):
                        sp0 = psum1024()
                        sp1 = psum1024()
                        gate = None
                        last = None
                        for j in range(2):
                            ktile = sk * 2 + j
                            ksl = slice(ktile * P, (ktile + 1) * P)
                            jsl = slice(j * 512, (j + 1) * 512)
                            a = mm(sp0[:, jsl], kt[0:64, ksl], qt[0:64, qsl])
                            b = mm(sp1[:, jsl], kt[64:128, ksl],
                                   qt[64:128, qsl])
                            # pin the exact PE order h0,h64,h0,h64 so the
                            # row-strip pairs stream concurrently
                            _add_dep_helper(b.ins, a.ins, sync=False,
                                            reason="pair order")
                            if last is not None:
                                _add_dep_helper(a.ins, last.ins, sync=False,
                                                reason="pair order")
                            last = b
                            if j == 0:
                                gate = b
                        # A@V lags two super-k's behind the scores so its
                        # exp() inputs are always long done.
                        if len(pending) >= 2:
                            psk, pea0, pea1 = pending.pop(0)
                            emit_av(psk, pea0, pea1, gate)
                        ea0 = e_pool.tile([P, 1024], DTM, tag="ea")
                        ea1 = e_pool.tile([P, 1024], DTM, tag="ea")
                        nc.scalar.activation(ea0[:], sp0[:], EXP, scale=0.125)
                        nc.scalar.activation(ea1[:], sp1[:], EXP, scale=0.125)
                        pending.append((sk, ea0, ea1))
                    for psk, pea0, pea1 in pending:
                        emit_av(psk, pea0, pea1, None)
                    # normalize: O[0:64] * (1 / O[64]) broadcast down.
                    # Copy O out of PSUM immediately (frees the bank),
                    # then run the denominator chain out of SBUF.
                    for o_ps, ot in ((o0, ot0), (o1, ot1)):
                        osb = rc_pool.tile([65, 512], F32, tag="osb")
                        nc.vector.tensor_copy(out=osb[:], in_=o_ps[:])
                        bc = psum512()
                        mm(bc[0:64, :], ones64_sb[64:65, :], osb[64:65, :])
                        rbc = rc_pool.tile([64, 512], F32, tag="rbc")
                        nc.vector.reciprocal(out=rbc[:], in_=bc[0:64, :])
                        nc.vector.tensor_mul(ot[:, qsl], osb[0:64, :], rbc[:])

        # ---- stage D: output projection Y = concat_h(O_h) @ Wo + bo ------
        with (
            tc.tile_pool(name="wo", bufs=8) as wo_pool,
            tc.tile_pool(name="y", bufs=2) as y_pool,
        ):
            wo_sb = []
            for h in range(H):
                wst = y_pool.tile([64, D], F32, tag="wostg")
                nc.sync.dma_start(out=wst[:], in_=wo[h * 64:(h + 1) * 64, :])
                woh = wo_pool.tile([64, D], DTM, tag="wo")
                nc.vector.tensor_copy(out=woh[:], in_=wst[:])
                wo_sb.append(woh)
            for qt_i in range(QL // P):
                ps = psum512()
                for h in range(H):
                    mm(ps[:], OT[h][:, qt_i * P:(qt_i + 1) * P], wo_sb[h][:],
                       start=(h == 0), stop=False)
                mm(ps[:], ones_sb[0:1, 0:P], bias_sb["bo"][0:1, :],
                   start=False, stop=True)
                ysb = y_pool.tile([P, D], F32, tag="y")
                nc.vector.tensor_copy(out=ysb[:], in_=ps[:])
                nc.sync.dma_start(out=out[qt_i * P:(qt_i + 1) * P, :], in_=ysb[:])


def build():
    nc = bacc.Bacc("TRN2", target_bir_lowering=False, debug=False,
                   num_devices=N_CORES)
    io = {}
    for nm, shape in (("xb", [S, D]), ("xq", [QL, D]), ("wq", [D, D]),
                      ("wk", [D, D]), ("wv", [D, D]), ("wo", [D, D]),
                      ("bq", [1, D]), ("bk", [1, D]), ("bv", [1, D]),
                      ("bo", [1, D]), ("ident", [P, P])):
        io[nm] = nc.dram_tensor(nm, shape, F32, kind="ExternalInput").ap()
    io["out"] = nc.dram_tensor("out", [QL, D], F32, kind="ExternalOutput").ap()
    with tile.TileContext(nc) as tc:
        with ExitStack() as ctx:
            _emit(ctx, tc, io)
    nc.compile()
    return nc


def make_in_maps(inputs):
    f = lambda a: np.ascontiguousarray(np.asarray(a, dtype=np.float32))
    x = f(inputs["x"])
    fixed = {
        "wq": f(inputs["Wq"]), "wk": f(inputs["Wk"]), "wv": f(inputs["Wv"]),
        "wo": f(inputs["Wo"]),
        "bq": f(inputs["bq"]).reshape(1, D), "bk": f(inputs["bk"]).reshape(1, D),
        "bv": f(inputs["bv"]).reshape(1, D), "bo": f(inputs["bo"]).reshape(1, D),
        "ident": np.eye(P, dtype=np.float32),
    }
    in_maps = []
    for c in range(N_CORES):
        b, qs = c // 4, (c % 4) * QL
        in_maps.append({"xb": x[b], "xq": x[b, qs:qs + QL], **fixed})
    return in_maps


_CACHE = {}
LAST_EXEC_NS = None


def run(inputs, trace=False):
    global LAST_EXEC_NS
    if "nc" not in _CACHE:
        _CACHE["nc"] = build()
    nc = _CACHE["nc"]
    kw = {}
    if trace:
        import sys, types
        if "antenv.axon_hooks" not in sys.modules:
            sys.path.insert(0, "/root/.axon_site")
            try:
                from trn_agent_boot.trn_boot import _ntff_profile_via_ctypes
                hook = _ntff_profile_via_ctypes("/opt/axon/libaxon_pjrt.so")
                mod = types.ModuleType("antenv.axon_hooks")
                mod.get_axon_ntff_profile_hook = lambda: hook
                mod.set_axon_ntff_profile_hook = lambda h: None
                sys.modules["antenv.axon_hooks"] = mod
            except Exception:
                pass
        kw = dict(trace=True, trace_cores=[0])
    res = run_bass_kernel_spmd(nc, make_in_maps(inputs),
                               core_ids=list(range(N_CORES)), **kw)
    if trace:
        LAST_EXEC_NS = res.exec_time_ns
    out = np.empty((B, S, D), np.float32)
    for c in range(N_CORES):
        b, qs = c // 4, (c % 4) * QL
        out[b, qs:qs + QL] = res.results[c]["out"]
    return out


def kernel(**inputs) -> np.ndarray:
    return run(inputs, trace=False)


# revision 17
# speedup vs baseline: 1.6629x; 1.0153x over previous
"""Multi-head self-attention Trainium2 Bass kernel (8-core SPMD).

Sharding: data-parallel over query rows. The flattened (B*S, D) = (8192, 512)
query space is split into 8 blocks of 1024 rows; core c handles batch c//4,
query rows (c%4)*1024 .. +1024. Each core recomputes K/V for its whole batch
(4-way duplicated) which avoids any cross-core communication; host-side
gather is a pure concatenation.

Layout strategy: activations live transposed in SBUF ([D, S], d on
partitions). Projections then need no weight transposes:
  K^T = Wk^T x^T   (lhsT = Wk chunk, rhs = x^T chunk)
  V   = x Wv       (lhsT = x^T chunk, rhs = Wv chunk)
Scores are computed transposed ([k, q], k on partitions) so softmax's
denominator comes from a ones-column appended to V (row 64 of the attention
output accumulator), and A^T is directly consumable by the A@V matmul.
exp() runs on the scalar engine with the 1/sqrt(dk) folded into its scale.
The normalized per-head outputs O^T are exactly the lhsT the output
projection wants, so no transposes are needed anywhere except on the input x.

Matmul operands are stored as fp16 (10-bit mantissa; measured end-to-end
absmax relative error ~4e-4): this is the true MAC path, so the PE
clock-gate can warm to 2.4 GHz and fast weight load applies. All
accumulation is fp32 in PSUM; softmax denominators/reciprocals are fp32.

Emission order is arranged to minimize startup serialization: xq is
transposed before xb so Q^T projections overlap the x^T transposes; K^T for
pair 0 precedes the V projection, which is split in head-halves so the
first attention block starts as early as possible. A@V matmuls lag two
k-super-tiles behind the scores, and scheduler dependency edges pin the
paired heads' score matmuls adjacent so they stream through disjoint PE
row strips concurrently.
"""

from contextlib import ExitStack

import numpy as np

import concourse.bass as bass
import concourse.tile as tile
from concourse import bacc, mybir
from concourse.bass import _add_dep_helper
from concourse.bass_utils import run_bass_kernel_spmd

N_CORES = 8
B, S, D, H, DK = 2, 4096, 512, 8, 64
QL = B * S // N_CORES          # 1024 query rows per core
P = 128
NT_S = S // P                  # 32 sequence tiles
NT_D = D // P                  # 4 d-model chunks
QC = QL // 512                 # 2 query chunks of 512
VW = 4 * 65                    # 260: per-k-tile width of an augmented V half
F32 = mybir.dt.float32
F32R = mybir.dt.float32r
F16 = mybir.dt.float16
EXP = mybir.ActivationFunctionType.Exp

# "f16" (10 mantissa bits, 2.4 GHz MAC path + FWL), "f32r" (13 bits but
# pinned at the 1.2 GHz throttled clock), "f32" (exact, 4 cycles/row).
MM_DTYPE = "f16"
DTM = {"f32r": F32R, "f16": F16, "f32": F32}[MM_DTYPE]


def _emit(ctx: ExitStack, tc: tile.TileContext, io: dict):
    nc = tc.nc
    xb, xq = io["xb"], io["xq"]
    wq, wk, wv, wo = io["wq"], io["wk"], io["wv"], io["wo"]
    bq, bk, bv, bo = io["bq"], io["bk"], io["bv"], io["bo"]
    ident = io["ident"]
    out = io["out"]

    mm = nc.tensor.matmul

    # ---- pools persistent across the whole kernel ------------------------
    consts = ctx.enter_context(tc.tile_pool(name="consts", bufs=1))
    xt_pool = ctx.enter_context(tc.tile_pool(name="xt", bufs=1))
    qt_pool = ctx.enter_context(tc.tile_pool(name="qt", bufs=4))
    v_pool = ctx.enter_context(tc.tile_pool(name="v", bufs=2))
    # PSUM: shared [128,1024] pool (3 bufs x 2 banks) + attention
    # accumulators (2 banks). Projections use [0:512] slices of the pool.
    ps_pool = ctx.enter_context(tc.tile_pool(name="ps", bufs=3, space="PSUM"))
    o_pool = ctx.enter_context(tc.tile_pool(name="o", bufs=2, space="PSUM"))

    def psum1024(dt=F32):
        return ps_pool.tile([P, 1024], dt, tag="ps", name="ps")

    def psum512(dt=F32):
        return psum1024(dt)[:, 0:512]

    # ---- constants --------------------------------------------------------
    ident_sb = consts.tile([P, P], F32, tag="ident")
    nc.sync.dma_start(out=ident_sb[:], in_=ident[:])
    ones_f32 = consts.tile([P, 1], F32, tag="ones_f32")
    nc.vector.memset(ones_f32[:], 1.0)
    ones_sb = consts.tile([1, 512], DTM, tag="ones")
    nc.vector.tensor_copy(out=ones_sb[:], in_=ones_f32[0:1, 0:1].broadcast_to([1, 512]))
    # a f32 ones row living on partition 64 (denominator broadcast lhsT)
    ones64_sb = consts.tile([65, 64], F32, tag="ones64")
    nc.vector.memset(ones64_sb[64:65, :], 1.0)
    # biases: bv/bo as [1, D] rows (free-dim biases, applied via rank-1
    # matmuls); bk/bq as [128, 4] per-partition columns (fused into the
    # PSUM->SBUF copies on the vector engine).
    bias_sb = {}
    with tc.tile_pool(name="stg0", bufs=2) as stg0:
        for nm, ap in (("bv", bv), ("bo", bo)):
            st = stg0.tile([1, D], F32, tag="bstg")
            nc.sync.dma_start(out=st[:], in_=ap[:])
            t = consts.tile([1, D], DTM, tag=nm)
            nc.vector.tensor_copy(out=t[:], in_=st[:])
            bias_sb[nm] = t
    bkT = consts.tile([P, 4], F32, tag="bkT")
    nc.sync.dma_start(out=bkT[:], in_=bk.rearrange("o (pr p) -> p (o pr)", p=P))
    bqT = consts.tile([P, 4], F32, tag="bqT")
    nc.sync.dma_start(out=bqT[:], in_=bq.rearrange("o (pr p) -> p (o pr)", p=P))

    xT = xt_pool.tile([P, NT_D * S], DTM, tag="xT")
    wq_r = wq.rearrange("(dc p) m -> p dc m", p=P)
    wk_r = wk.rearrange("(dc p) m -> p dc m", p=P)
    wv_r = wv.rearrange("(dc p) m -> p dc m", p=P)
    QT = []
    OT = []  # per-head [64, QL] normalized attention outputs (transposed)

    ot_pool = ctx.enter_context(tc.tile_pool(name="ot", bufs=8))
    wkv_pool = ctx.enter_context(tc.tile_pool(name="wkv", bufs=2))
    stgc = ctx.enter_context(tc.tile_pool(name="stgc", bufs=2))
    kt_pool = ctx.enter_context(tc.tile_pool(name="kt", bufs=1))
    e_pool = ctx.enter_context(tc.tile_pool(name="e", bufs=6))
    rc_pool = ctx.enter_context(tc.tile_pool(name="rc", bufs=4))

    def transpose_in(src_ap, ntile, dst, xn_pool):
        """DMA f32 rows, PE-transpose 128x128 blocks, convert-copy to dst."""
        for st in range(ntile):
            xn = xn_pool.tile([P, D], F32, tag="xn")
            nc.sync.dma_start(out=xn[:], in_=src_ap[st * P:(st + 1) * P, :])
            tp = psum512()
            for dc in range(NT_D):
                nc.tensor.transpose(
                    tp[:, dc * P:(dc + 1) * P],
                    xn[:, dc * P:(dc + 1) * P],
                    ident_sb[:],
                )
            dst_ap = dst[:, :].rearrange("p (dc s) -> p dc s", dc=NT_D)
            nc.vector.tensor_copy(
                out=dst_ap[:, :, st * P:(st + 1) * P],
                in_=tp[:, :].rearrange("p (dc j) -> p dc j", dc=NT_D),
            )

    def emit_qt(pr, xqT, wq_pool, stgb):
        wst = stgb.tile([P, NT_D * P], F32, tag="wstg")
        nc.sync.dma_start(
            out=wst[:, :].rearrange("p (dc m) -> p dc m", dc=NT_D),
            in_=wq_r[:, :, pr * P:(pr + 1) * P],
        )
        wqp = wq_pool.tile([P, NT_D * P], DTM, tag="wq")
        nc.vector.tensor_copy(out=wqp[:], in_=wst[:])
        qt = qt_pool.tile([P, QL], DTM, tag="QT")
        for qc in range(QC):
            ps = psum512()
            for dc in range(NT_D):
                mm(ps[:], wqp[:, dc * P:(dc + 1) * P],
                   xqT[:, dc * QL + qc * 512:dc * QL + (qc + 1) * 512],
                   start=(dc == 0), stop=(dc == NT_D - 1))
            nc.vector.tensor_scalar_add(
                out=qt[:, qc * 512:(qc + 1) * 512], in0=ps[:],
                scalar1=bqT[:, pr:pr + 1],
            )
        QT.append(qt)

    def emit_kt(pr):
        wst = stgc.tile([P, NT_D * 256], F32, tag="wstg")
        nc.sync.dma_start(
            out=wst[:, 0:NT_D * P].rearrange("p (dc m) -> p dc m", dc=NT_D),
            in_=wk_r[:, :, pr * P:(pr + 1) * P],
        )
        wkp = wkv_pool.tile([P, NT_D * P], DTM, tag="wk")
        nc.vector.tensor_copy(out=wkp[:], in_=wst[:, 0:NT_D * P])
        kt = kt_pool.tile([P, S], DTM, tag="KT")
        for sc in range(8):
            ps = psum512()
            for dc in range(NT_D):
                mm(ps[:], wkp[:, dc * P:(dc + 1) * P],
                   xT[:, dc * S + sc * 512:dc * S + (sc + 1) * 512],
                   start=(dc == 0), stop=(dc == NT_D - 1))
            nc.vector.tensor_scalar_add(
                out=kt[:, sc * 512:(sc + 1) * 512], in0=ps[:],
                scalar1=bkT[:, pr:pr + 1],
            )
        return kt

    def emit_v_half(hf):
        """V for heads 4*hf..4*hf+3 with a ones column per head:
        vaug[:, kt*260 + hl*65 + (0..63)] = V[k-tile kt, head 4*hf+hl]
        vaug[:, kt*260 + hl*65 + 64]      = 1.0
        """
        wst = stgc.tile([P, NT_D * 256], F32, tag="wstg")
        nc.sync.dma_start(
            out=wst[:, :].rearrange("p (dc m) -> p dc m", dc=NT_D),
            in_=wv_r[:, :, hf * 256:(hf + 1) * 256],
        )
        wvg = wkv_pool.tile([P, NT_D * 256], DTM, tag="wv")
        nc.vector.tensor_copy(out=wvg[:], in_=wst[:])
        vaug = v_pool.tile([P, NT_S * VW], DTM, tag="vaug")
        nc.vector.tensor_copy(
            out=vaug[:, :].rearrange("p (t h e) -> p t h e",
                                     t=NT_S, h=4)[:, :, :, 64:65],
            in_=ones_f32[:, 0:1].broadcast_to([P, NT_S, 4, 1]),
        )
        for st in range(NT_S):
            ps = psum512()
            for dc in range(NT_D):
                mm(ps[:, 0:256], xT[:, dc * S + st * P:dc * S + (st + 1) * P],
                   wvg[:, dc * 256:(dc + 1) * 256],
                   start=(dc == 0), stop=False)
            mm(ps[:, 0:256], ones_sb[0:1, 0:P],
               bias_sb["bv"][0:1, hf * 256:(hf + 1) * 256],
               start=False, stop=True)
            dst = vaug[:, st * VW:(st + 1) * VW]
            dst = dst.rearrange("p (h e) -> p h e", h=4)[:, :, 0:64]
            nc.vector.tensor_copy(
                out=dst, in_=ps[:, 0:256].rearrange("p (h e) -> p h e", h=4)
            )
        return vaug

    def emit_attention(pr, kt, vaug, qt):
        hl0, hl1 = (pr % 2) * 2, (pr % 2) * 2 + 1
        ot0 = ot_pool.tile([64, QL], DTM, tag="OT")
        ot1 = ot_pool.tile([64, QL], DTM, tag="OT")
        OT.extend([ot0, ot1])
        for qc in range(QC):
            qsl = slice(qc * 512, (qc + 1) * 512)
            o0 = o_pool.tile([65, 512], F32, tag="O")
            o1 = o_pool.tile([65, 512], F32, tag="O")

            def emit_av(sk, ea0, ea1, gate):
                for j in range(2):
                    ktile = sk * 2 + j
                    st_ = ktile * VW
                    esl = slice(j * 512, (j + 1) * 512)
                    fl = dict(start=(ktile == 0), stop=(ktile == NT_S - 1))
                    i0 = mm(o0[:], vaug[:, st_ + hl0 * 65:st_ + hl0 * 65 + 65],
                            ea0[:, esl], **fl)
                    i1 = mm(o1[:], vaug[:, st_ + hl1 * 65:st_ + hl1 * 65 + 65],
                            ea1[:, esl], **fl)
                    if gate is not None:
                        # order A@V after the next score pair: keeps the
                        # paired heads adjacent in the PE stream
                        _add_dep_helper(i0.ins, gate.ins, sync=False,
                                        reason="attn pipeline order")
                        _add_dep_helper(i1.ins, gate.ins, sync=False,
                                        reason="attn pipeline order")

            pending = []  # [(sk, ea0, ea1), ...] not yet AV-emitted
            for sk in range(NT_S // 2):
                sp0 = psum1024()
                sp1 = psum1024()
                gate = None
                last = None
                for j in range(2):
                    ktile = sk * 2 + j
                    ksl = slice(ktile * P, (ktile + 1) * P)
                    jsl = slice(j * 512, (j + 1) * 512)
                    a = mm(sp0[:, jsl], kt[0:64, ksl], qt[0:64, qsl])
                    b = mm(sp1[:, jsl], kt[64:128, ksl], qt[64:128, qsl])
                    # pin the exact PE order h0,h64,h0,h64 so the row-strip
                    # pairs stream concurrently
                    _add_dep_helper(b.ins, a.ins, sync=False,
                                    reason="pair order")
                    if last is not None:
                        _add_dep_helper(a.ins, last.ins, sync=False,
                                        reason="pair order")
                    last = b
                    if j == 0:
                        gate = b
                # A@V lags two super-k's behind the scores so its exp()
                # inputs are always long done.
                if len(pending) >= 2:
                    psk, pea0, pea1 = pending.pop(0)
                    emit_av(psk, pea0, pea1, gate)
                ea0 = e_pool.tile([P, 1024], DTM, tag="ea")
                ea1 = e_pool.tile([P, 1024], DTM, tag="ea")
                nc.scalar.activation(ea0[:], sp0[:], EXP, scale=0.125)
                nc.scalar.activation(ea1[:], sp1[:], EXP, scale=0.125)
                pending.append((sk, ea0, ea1))
            for psk, pea0, pea1 in pending:
                emit_av(psk, pea0, pea1, None)
            # normalize: O[0:64] * (1 / O[64]) broadcast down. Copy O out
            # of PSUM immediately (frees the bank), then run the
            # denominator chain out of SBUF.
            for o_ps, ot in ((o0, ot0), (o1, ot1)):
                osb = rc_pool.tile([65, 512], F32, tag="osb")
                nc.vector.tensor_copy(out=osb[:], in_=o_ps[:])
                bc = psum512()
                mm(bc[0:64, :], ones64_sb[64:65, :], osb[64:65, :])
                rbc = rc_pool.tile([64, 512], F32, tag="rbc")
                nc.vector.reciprocal(out=rbc[:], in_=bc[0:64, :])
                nc.vector.tensor_mul(ot[:, qsl], osb[0:64, :], rbc[:])

    # ---- stages ----------------------------------------------------------
    with tc.tile_pool(name="xq", bufs=1) as xq_pool:
        xqT = xq_pool.tile([P, NT_D * QL], DTM, tag="xqT")
        with (
            tc.tile_pool(name="xn", bufs=6) as xn_pool,
            tc.tile_pool(name="wqp", bufs=2) as wq_pool,
            tc.tile_pool(name="stgb", bufs=2) as stgb,
        ):
            transpose_in(xq, QL // P, xqT, xn_pool)     # xq^T first
            emit_qt(0, xqT, wq_pool, stgb)              # Q^T pair 0 asap
            transpose_in(xb, NT_S, xT, xn_pool)         # x^T
            for pr in range(1, 4):
                emit_qt(pr, xqT, wq_pool, stgb)

    kt0 = emit_kt(0)
    va0 = emit_v_half(0)                                # heads 0-3
    emit_attention(0, kt0, va0, QT[0])
    kt1 = emit_kt(1)
    emit_attention(1, kt1, va0, QT[1])
    va1 = emit_v_half(1)                                # heads 4-7
    kt2 = emit_kt(2)
    emit_attention(2, kt2, va1, QT[2])
    kt3 = emit_kt(3)
    emit_attention(3, kt3, va1, QT[3])

    # ---- stage D: output projection Y = concat_h(O_h) @ Wo + bo ----------
    with (
        tc.tile_pool(name="wo", bufs=8) as wo_pool,
        tc.tile_pool(name="y", bufs=2) as y_pool,
    ):
        wo_sb = []
        for h in range(H):
            wst = y_pool.tile([64, D], F32, tag="wostg")
            nc.sync.dma_start(out=wst[:], in_=wo[h * 64:(h + 1) * 64, :])
            woh = wo_pool.tile([64, D], DTM, tag="wo")
            nc.vector.tensor_copy(out=woh[:], in_=wst[:])
            wo_sb.append(woh)
        for qt_i in range(QL // P):
            ps = psum512()
            for h in range(H):
                mm(ps[:], OT[h][:, qt_i * P:(qt_i + 1) * P], wo_sb[h][:],
                   start=(h == 0), stop=False)
            mm(ps[:], ones_sb[0:1, 0:P], bias_sb["bo"][0:1, :],
               start=False, stop=True)
            ysb = y_pool.tile([P, D], F32, tag="y")
            nc.vector.tensor_copy(out=ysb[:], in_=ps[:])
            nc.sync.dma_start(out=out[qt_i * P:(qt_i + 1) * P, :], in_=ysb[:])


def build():
    nc = bacc.Bacc("TRN2", target_bir_lowering=False, debug=False,
                   num_devices=N_CORES)
    io = {}
    for nm, shape in (("xb", [S, D]), ("xq", [QL, D]), ("wq", [D, D]),
                      ("wk", [D, D]), ("wv", [D, D]), ("wo", [D, D]),
                      ("bq", [1, D]), ("bk", [1, D]), ("bv", [1, D]),
                      ("bo", [1, D]), ("ident", [P, P])):
        io[nm] = nc.dram_tensor(nm, shape, F32, kind="ExternalInput").ap()
    io["out"] = nc.dram_tensor("out", [QL, D], F32, kind="ExternalOutput").ap()
    with tile.TileContext(nc) as tc:
        with ExitStack() as ctx:
            _emit(ctx, tc, io)
    nc.compile()
    return nc


def make_in_maps(inputs):
    f = lambda a: np.ascontiguousarray(np.asarray(a, dtype=np.float32))
    x = f(inputs["x"])
    fixed = {
        "wq": f(inputs["Wq"]), "wk": f(inputs["Wk"]), "wv": f(inputs["Wv"]),
        "wo": f(inputs["Wo"]),
        "bq": f(inputs["bq"]).reshape(1, D), "bk": f(inputs["bk"]).reshape(1, D),
        "bv": f(inputs["bv"]).reshape(1, D), "bo": f(inputs["bo"]).reshape(1, D),
        "ident": np.eye(P, dtype=np.float32),
    }
    in_maps = []
    for c in range(N_CORES):
        b, qs = c // 4, (c % 4) * QL
        in_maps.append({"xb": x[b], "xq": x[b, qs:qs + QL], **fixed})
    return in_maps


_CACHE = {}
LAST_EXEC_NS = None


def run(inputs, trace=False):
    global LAST_EXEC_NS
    if "nc" not in _CACHE:
        _CACHE["nc"] = build()
    nc = _CACHE["nc"]
    kw = {}
    if trace:
        import sys, types
        if "antenv.axon_hooks" not in sys.modules:
            sys.path.insert(0, "/root/.axon_site")
            try:
                from trn_agent_boot.trn_boot import _ntff_profile_via_ctypes
                hook = _ntff_profile_via_ctypes("/opt/axon/libaxon_pjrt.so")
                mod = types.ModuleType("antenv.axon_hooks")
                mod.get_axon_ntff_profile_hook = lambda: hook
                mod.set_axon_ntff_profile_hook = lambda h: None
                sys.modules["antenv.axon_hooks"] = mod
            except Exception:
                pass
        kw = dict(trace=True, trace_cores=[0])
    res = run_bass_kernel_spmd(nc, make_in_maps(inputs),
                               core_ids=list(range(N_CORES)), **kw)
    if trace:
        LAST_EXEC_NS = res.exec_time_ns
    out = np.empty((B, S, D), np.float32)
    for c in range(N_CORES):
        b, qs = c // 4, (c % 4) * QL
        out[b, qs:qs + QL] = res.results[c]["out"]
    return out


def kernel(**inputs) -> np.ndarray:
    return run(inputs, trace=False)


# revision 18
# speedup vs baseline: 1.6701x; 1.0043x over previous
"""Multi-head self-attention Trainium2 Bass kernel (8-core SPMD).

Sharding: data-parallel over query rows. The flattened (B*S, D) = (8192, 512)
query space is split into 8 blocks of 1024 rows; core c handles batch c//4,
query rows (c%4)*1024 .. +1024. Each core recomputes K/V for its whole batch
(4-way duplicated) which avoids any cross-core communication; host-side
gather is a pure concatenation.

Layout strategy: activations live transposed in SBUF ([D, S], d on
partitions). Projections then need no weight transposes:
  K^T = Wk^T x^T   (lhsT = Wk chunk, rhs = x^T chunk)
  V   = x Wv       (lhsT = x^T chunk, rhs = Wv chunk)
Scores are computed transposed ([k, q], k on partitions) so softmax's
denominator comes from a ones-column appended to V (row 64 of the attention
output accumulator), and A^T is directly consumable by the A@V matmul.
exp() runs on the scalar engine with the 1/sqrt(dk) folded into its scale.
The normalized per-head outputs O^T are exactly the lhsT the output
projection wants, so no transposes are needed anywhere except on the input x.

Matmul operands are stored as fp16 (10-bit mantissa; measured end-to-end
absmax relative error ~4e-4): this is the true MAC path, so the PE
clock-gate can warm to 2.4 GHz and fast weight load applies. All
accumulation is fp32 in PSUM; softmax denominators/reciprocals are fp32.

Emission order is arranged to minimize startup serialization: xq is
transposed before xb so Q^T projections overlap the x^T transposes; K^T for
pair 0 precedes the V projection, which is split in head-halves so the
first attention block starts as early as possible. A@V matmuls lag two
k-super-tiles behind the scores, and scheduler dependency edges pin the
paired heads' score matmuls adjacent so they stream through disjoint PE
row strips concurrently.
"""

from contextlib import ExitStack

import numpy as np

import concourse.bass as bass
import concourse.tile as tile
from concourse import bacc, mybir
from concourse.bass import _add_dep_helper
from concourse.bass_utils import run_bass_kernel_spmd

N_CORES = 8
B, S, D, H, DK = 2, 4096, 512, 8, 64
QL = B * S // N_CORES          # 1024 query rows per core
P = 128
NT_S = S // P                  # 32 sequence tiles
NT_D = D // P                  # 4 d-model chunks
QC = QL // 512                 # 2 query chunks of 512
VW = 4 * 65                    # 260: per-k-tile width of an augmented V half
F32 = mybir.dt.float32
F32R = mybir.dt.float32r
F16 = mybir.dt.float16
EXP = mybir.ActivationFunctionType.Exp

# "f16" (10 mantissa bits, 2.4 GHz MAC path + FWL), "f32r" (13 bits but
# pinned at the 1.2 GHz throttled clock), "f32" (exact, 4 cycles/row).
MM_DTYPE = "f16"
DTM = {"f32r": F32R, "f16": F16, "f32": F32}[MM_DTYPE]


def _emit(ctx: ExitStack, tc: tile.TileContext, io: dict):
    nc = tc.nc
    xb, xq = io["xb"], io["xq"]
    wq, wk, wv, wo = io["wq"], io["wk"], io["wv"], io["wo"]
    bq, bk, bv, bo = io["bq"], io["bk"], io["bv"], io["bo"]
    ident = io["ident"]
    out = io["out"]

    mm = nc.tensor.matmul

    # ---- pools persistent across the whole kernel ------------------------
    consts = ctx.enter_context(tc.tile_pool(name="consts", bufs=1))
    xt_pool = ctx.enter_context(tc.tile_pool(name="xt", bufs=1))
    qt_pool = ctx.enter_context(tc.tile_pool(name="qt", bufs=4))
    v_pool = ctx.enter_context(tc.tile_pool(name="v", bufs=2))
    # PSUM: shared [128,1024] pool (3 bufs x 2 banks) + attention
    # accumulators (2 banks). Projections use [0:512] slices of the pool.
    ps_pool = ctx.enter_context(tc.tile_pool(name="ps", bufs=3, space="PSUM"))
    o_pool = ctx.enter_context(tc.tile_pool(name="o", bufs=2, space="PSUM"))

    def psum1024(dt=F32):
        return ps_pool.tile([P, 1024], dt, tag="ps", name="ps")

    def psum512(dt=F32):
        return psum1024(dt)[:, 0:512]

    # ---- constants --------------------------------------------------------
    ident_sb = consts.tile([P, P], F32, tag="ident")
    nc.sync.dma_start(out=ident_sb[:], in_=ident[:])
    ones_f32 = consts.tile([P, 1], F32, tag="ones_f32")
    nc.vector.memset(ones_f32[:], 1.0)
    ones_sb = consts.tile([1, 512], DTM, tag="ones")
    nc.vector.tensor_copy(out=ones_sb[:], in_=ones_f32[0:1, 0:1].broadcast_to([1, 512]))
    # a f32 ones row living on partition 64 (denominator broadcast lhsT)
    ones64_sb = consts.tile([65, 64], F32, tag="ones64")
    nc.vector.memset(ones64_sb[64:65, :], 1.0)
    # biases: bv/bo as [1, D] rows (free-dim biases, applied via rank-1
    # matmuls); bk/bq as [128, 4] per-partition columns (fused into the
    # PSUM->SBUF copies on the vector engine).
    bias_sb = {}
    with tc.tile_pool(name="stg0", bufs=2) as stg0:
        for nm, ap in (("bv", bv), ("bo", bo)):
            st = stg0.tile([1, D], F32, tag="bstg")
            nc.sync.dma_start(out=st[:], in_=ap[:])
            t = consts.tile([1, D], DTM, tag=nm)
            nc.vector.tensor_copy(out=t[:], in_=st[:])
            bias_sb[nm] = t
    bkT = consts.tile([P, 4], F32, tag="bkT")
    nc.sync.dma_start(out=bkT[:], in_=bk.rearrange("o (pr p) -> p (o pr)", p=P))
    bqT = consts.tile([P, 4], F32, tag="bqT")
    nc.sync.dma_start(out=bqT[:], in_=bq.rearrange("o (pr p) -> p (o pr)", p=P))

    xT = xt_pool.tile([P, NT_D * S], DTM, tag="xT")
    wq_r = wq.rearrange("(dc p) m -> p dc m", p=P)
    wk_r = wk.rearrange("(dc p) m -> p dc m", p=P)
    wv_r = wv.rearrange("(dc p) m -> p dc m", p=P)
    QT = []
    OT = []  # per-head [64, QL] normalized attention outputs (transposed)

    ot_pool = ctx.enter_context(tc.tile_pool(name="ot", bufs=8))
    wkv_pool = ctx.enter_context(tc.tile_pool(name="wkv", bufs=2))
    stgc = ctx.enter_context(tc.tile_pool(name="stgc", bufs=2))
    kt_pool = ctx.enter_context(tc.tile_pool(name="kt", bufs=1))
    e_pool = ctx.enter_context(tc.tile_pool(name="e", bufs=6))
    rc_pool = ctx.enter_context(tc.tile_pool(name="rc", bufs=4))

    def transpose_in(src_ap, ntile, dst, xn_pool):
        """DMA f32 rows, PE-transpose 128x128 blocks, convert-copy to dst."""
        for st in range(ntile):
            xn = xn_pool.tile([P, D], F32, tag="xn")
            nc.sync.dma_start(out=xn[:], in_=src_ap[st * P:(st + 1) * P, :])
            tp = psum512()
            for dc in range(NT_D):
                nc.tensor.transpose(
                    tp[:, dc * P:(dc + 1) * P],
                    xn[:, dc * P:(dc + 1) * P],
                    ident_sb[:],
                )
            dst_ap = dst[:, :].rearrange("p (dc s) -> p dc s", dc=NT_D)
            nc.vector.tensor_copy(
                out=dst_ap[:, :, st * P:(st + 1) * P],
                in_=tp[:, :].rearrange("p (dc j) -> p dc j", dc=NT_D),
            )

    def emit_qt(pr, xqT, wq_pool, stgb):
        wst = stgb.tile([P, NT_D * P], F32, tag="wstg")
        nc.sync.dma_start(
            out=wst[:, :].rearrange("p (dc m) -> p dc m", dc=NT_D),
            in_=wq_r[:, :, pr * P:(pr + 1) * P],
        )
        wqp = wq_pool.tile([P, NT_D * P], DTM, tag="wq")
        nc.vector.tensor_copy(out=wqp[:], in_=wst[:])
        qt = qt_pool.tile([P, QL], DTM, tag="QT")
        for qc in range(QC):
            ps = psum512()
            for dc in range(NT_D):
                mm(ps[:], wqp[:, dc * P:(dc + 1) * P],
                   xqT[:, dc * QL + qc * 512:dc * QL + (qc + 1) * 512],
                   start=(dc == 0), stop=(dc == NT_D - 1))
            nc.vector.tensor_scalar_add(
                out=qt[:, qc * 512:(qc + 1) * 512], in0=ps[:],
                scalar1=bqT[:, pr:pr + 1],
            )
        QT.append(qt)

    def emit_kt(pr):
        wst = stgc.tile([P, NT_D * 256], F32, tag="wstg")
        nc.sync.dma_start(
            out=wst[:, 0:NT_D * P].rearrange("p (dc m) -> p dc m", dc=NT_D),
            in_=wk_r[:, :, pr * P:(pr + 1) * P],
        )
        wkp = wkv_pool.tile([P, NT_D * P], DTM, tag="wk")
        nc.vector.tensor_copy(out=wkp[:], in_=wst[:, 0:NT_D * P])
        kt = kt_pool.tile([P, S], DTM, tag="KT")
        for sc in range(8):
            ps = psum512()
            for dc in range(NT_D):
                mm(ps[:], wkp[:, dc * P:(dc + 1) * P],
                   xT[:, dc * S + sc * 512:dc * S + (sc + 1) * 512],
                   start=(dc == 0), stop=(dc == NT_D - 1))
            nc.vector.tensor_scalar_add(
                out=kt[:, sc * 512:(sc + 1) * 512], in0=ps[:],
                scalar1=bkT[:, pr:pr + 1],
            )
        return kt

    def emit_v_half(hf):
        """V for heads 4*hf..4*hf+3 with a ones column per head:
        vaug[:, kt*260 + hl*65 + (0..63)] = V[k-tile kt, head 4*hf+hl]
        vaug[:, kt*260 + hl*65 + 64]      = 1.0
        """
        wst = stgc.tile([P, NT_D * 256], F32, tag="wstg")
        nc.sync.dma_start(
            out=wst[:, :].rearrange("p (dc m) -> p dc m", dc=NT_D),
            in_=wv_r[:, :, hf * 256:(hf + 1) * 256],
        )
        wvg = wkv_pool.tile([P, NT_D * 256], DTM, tag="wv")
        nc.vector.tensor_copy(out=wvg[:], in_=wst[:])
        vaug = v_pool.tile([P, NT_S * VW], DTM, tag="vaug")
        nc.vector.tensor_copy(
            out=vaug[:, :].rearrange("p (t h e) -> p t h e",
                                     t=NT_S, h=4)[:, :, :, 64:65],
            in_=ones_f32[:, 0:1].broadcast_to([P, NT_S, 4, 1]),
        )
        for st in range(NT_S):
            ps = psum512()
            for dc in range(NT_D):
                mm(ps[:, 0:256], xT[:, dc * S + st * P:dc * S + (st + 1) * P],
                   wvg[:, dc * 256:(dc + 1) * 256],
                   start=(dc == 0), stop=False)
            mm(ps[:, 0:256], ones_sb[0:1, 0:P],
               bias_sb["bv"][0:1, hf * 256:(hf + 1) * 256],
               start=False, stop=True)
            dst = vaug[:, st * VW:(st + 1) * VW]
            dst = dst.rearrange("p (h e) -> p h e", h=4)[:, :, 0:64]
            nc.vector.tensor_copy(
                out=dst, in_=ps[:, 0:256].rearrange("p (h e) -> p h e", h=4)
            )
        return vaug

    def emit_attention(pr, kt, vaug, qt):
        hl0, hl1 = (pr % 2) * 2, (pr % 2) * 2 + 1
        ot0 = ot_pool.tile([64, QL], DTM, tag="OT")
        ot1 = ot_pool.tile([64, QL], DTM, tag="OT")
        OT.extend([ot0, ot1])
        for qc in range(QC):
            qsl = slice(qc * 512, (qc + 1) * 512)
            o0 = o_pool.tile([65, 512], F32, tag="O")
            o1 = o_pool.tile([65, 512], F32, tag="O")

            def emit_av(ktile, ea, gate):
                st_ = ktile * VW
                fl = dict(start=(ktile == 0), stop=(ktile == NT_S - 1))
                i0 = mm(o0[:], vaug[:, st_ + hl0 * 65:st_ + hl0 * 65 + 65],
                        ea[:, 0:512], **fl)
                i1 = mm(o1[:], vaug[:, st_ + hl1 * 65:st_ + hl1 * 65 + 65],
                        ea[:, 512:1024], **fl)
                if gate is not None:
                    # order A@V after the next score pair: keeps the
                    # paired heads adjacent in the PE stream
                    _add_dep_helper(i0.ins, gate.ins, sync=False,
                                    reason="attn pipeline order")
                    _add_dep_helper(i1.ins, gate.ins, sync=False,
                                    reason="attn pipeline order")

            pending = []  # [(ktile, ea), ...] not yet AV-emitted
            for ktile in range(NT_S):
                ksl = slice(ktile * P, (ktile + 1) * P)
                # both heads' scores share one [128,1024] PSUM tile
                sp = psum1024()
                a = mm(sp[:, 0:512], kt[0:64, ksl], qt[0:64, qsl])
                b = mm(sp[:, 512:1024], kt[64:128, ksl], qt[64:128, qsl])
                # pin h64 right after h0: the pair streams through
                # disjoint PE row strips concurrently
                _add_dep_helper(b.ins, a.ins, sync=False, reason="pair order")
                # A@V lags three k-tiles behind the scores so its exp()
                # inputs are always long done.
                if len(pending) >= 3:
                    pkt, pea = pending.pop(0)
                    emit_av(pkt, pea, b)
                ea = e_pool.tile([P, 1024], DTM, tag="ea")
                nc.scalar.activation(ea[:], sp[:], EXP, scale=0.125)
                pending.append((ktile, ea))
            for pkt, pea in pending:
                emit_av(pkt, pea, None)
            # normalize: O[0:64] * (1 / O[64]) broadcast down. Copy O out
            # of PSUM immediately (frees the bank), then run the
            # denominator chain out of SBUF.
            for o_ps, ot in ((o0, ot0), (o1, ot1)):
                osb = rc_pool.tile([65, 512], F32, tag="osb")
                nc.vector.tensor_copy(out=osb[:], in_=o_ps[:])
                bc = psum512()
                mm(bc[0:64, :], ones64_sb[64:65, :], osb[64:65, :])
                rbc = rc_pool.tile([64, 512], F32, tag="rbc")
                nc.vector.reciprocal(out=rbc[:], in_=bc[0:64, :])
                nc.vector.tensor_mul(ot[:, qsl], osb[0:64, :], rbc[:])

    # ---- stages ----------------------------------------------------------
    with tc.tile_pool(name="xq", bufs=1) as xq_pool:
        xqT = xq_pool.tile([P, NT_D * QL], DTM, tag="xqT")
        with (
            tc.tile_pool(name="xn", bufs=6) as xn_pool,
            tc.tile_pool(name="wqp", bufs=2) as wq_pool,
            tc.tile_pool(name="stgb", bufs=2) as stgb,
        ):
            transpose_in(xq, QL // P, xqT, xn_pool)     # xq^T first
            emit_qt(0, xqT, wq_pool, stgb)              # Q^T pair 0 asap
            transpose_in(xb, NT_S, xT, xn_pool)         # x^T
            for pr in range(1, 4):
                emit_qt(pr, xqT, wq_pool, stgb)

    kt0 = emit_kt(0)
    va0 = emit_v_half(0)                                # heads 0-3
    emit_attention(0, kt0, va0, QT[0])
    kt1 = emit_kt(1)
    emit_attention(1, kt1, va0, QT[1])
    va1 = emit_v_half(1)                                # heads 4-7
    kt2 = emit_kt(2)
    emit_attention(2, kt2, va1, QT[2])
    kt3 = emit_kt(3)
    emit_attention(3, kt3, va1, QT[3])

    # ---- stage D: output projection Y = concat_h(O_h) @ Wo + bo ----------
    with (
        tc.tile_pool(name="wo", bufs=8) as wo_pool,
        tc.tile_pool(name="y", bufs=2) as y_pool,
    ):
        wo_sb = []
        for h in range(H):
            wst = y_pool.tile([64, D], F32, tag="wostg")
            nc.sync.dma_start(out=wst[:], in_=wo[h * 64:(h + 1) * 64, :])
            woh = wo_pool.tile([64, D], DTM, tag="wo")
            nc.vector.tensor_copy(out=woh[:], in_=wst[:])
            wo_sb.append(woh)
        for qt_i in range(QL // P):
            ps = psum512()
            for h in range(H):
                mm(ps[:], OT[h][:, qt_i * P:(qt_i + 1) * P], wo_sb[h][:],
                   start=(h == 0), stop=False)
            mm(ps[:], ones_sb[0:1, 0:P], bias_sb["bo"][0:1, :],
               start=False, stop=True)
            ysb = y_pool.tile([P, D], F32, tag="y")
            nc.vector.tensor_copy(out=ysb[:], in_=ps[:])
            nc.sync.dma_start(out=out[qt_i * P:(qt_i + 1) * P, :], in_=ysb[:])


def build():
    nc = bacc.Bacc("TRN2", target_bir_lowering=False, debug=False,
                   num_devices=N_CORES)
    io = {}
    for nm, shape in (("xb", [S, D]), ("xq", [QL, D]), ("wq", [D, D]),
                      ("wk", [D, D]), ("wv", [D, D]), ("wo", [D, D]),
                      ("bq", [1, D]), ("bk", [1, D]), ("bv", [1, D]),
                      ("bo", [1, D]), ("ident", [P, P])):
        io[nm] = nc.dram_tensor(nm, shape, F32, kind="ExternalInput").ap()
    io["out"] = nc.dram_tensor("out", [QL, D], F32, kind="ExternalOutput").ap()
    with tile.TileContext(nc) as tc:
        with ExitStack() as ctx:
            _emit(ctx, tc, io)
    nc.compile()
    return nc


def make_in_maps(inputs):
    f = lambda a: np.ascontiguousarray(np.asarray(a, dtype=np.float32))
    x = f(inputs["x"])
    fixed = {
        "wq": f(inputs["Wq"]), "wk": f(inputs["Wk"]), "wv": f(inputs["Wv"]),
        "wo": f(inputs["Wo"]),
        "bq": f(inputs["bq"]).reshape(1, D), "bk": f(inputs["bk"]).reshape(1, D),
        "bv": f(inputs["bv"]).reshape(1, D), "bo": f(inputs["bo"]).reshape(1, D),
        "ident": np.eye(P, dtype=np.float32),
    }
    in_maps = []
    for c in range(N_CORES):
        b, qs = c // 4, (c % 4) * QL
        in_maps.append({"xb": x[b], "xq": x[b, qs:qs + QL], **fixed})
    return in_maps


_CACHE = {}
LAST_EXEC_NS = None


def run(inputs, trace=False):
    global LAST_EXEC_NS
    if "nc" not in _CACHE:
        _CACHE["nc"] = build()
    nc = _CACHE["nc"]
    kw = {}
    if trace:
        import sys, types
        if "antenv.axon_hooks" not in sys.modules:
            sys.path.insert(0, "/root/.axon_site")
            try:
                from trn_agent_boot.trn_boot import _ntff_profile_via_ctypes
                hook = _ntff_profile_via_ctypes("/opt/axon/libaxon_pjrt.so")
                mod = types.ModuleType("antenv.axon_hooks")
                mod.get_axon_ntff_profile_hook = lambda: hook
                mod.set_axon_ntff_profile_hook = lambda h: None
                sys.modules["antenv.axon_hooks"] = mod
            except Exception:
                pass
        kw = dict(trace=True, trace_cores=[0])
    res = run_bass_kernel_spmd(nc, make_in_maps(inputs),
                               core_ids=list(range(N_CORES)), **kw)
    if trace:
        LAST_EXEC_NS = res.exec_time_ns
    out = np.empty((B, S, D), np.float32)
    for c in range(N_CORES):
        b, qs = c // 4, (c % 4) * QL
        out[b, qs:qs + QL] = res.results[c]["out"]
    return out


def kernel(**inputs) -> np.ndarray:
    return run(inputs, trace=False)


# revision 20
# speedup vs baseline: 1.9475x; 1.1661x over previous
"""Multi-head self-attention Trainium2 Bass kernel (8-core SPMD).

Sharding: tensor-parallel over (batch, head-pair). With B=2 batches and
H=8 heads there are exactly 8 (batch, head-pair) units; core c handles
batch c//4 and heads {2*(c%4), 2*(c%4)+1}. Each core computes Q/K/V for its
two heads over the full sequence, runs attention, and produces the partial
output projection O_pair @ Wo_pair (no bias). The host sums the four
partials per batch and adds the output bias — a cheap numpy reduction.
Per-core weight slices are passed as separate inputs so the program stays
SPMD-uniform.

Layout strategy: activations live transposed in SBUF ([D, S], d on
partitions). Projections then need no weight transposes:
  K^T = Wk^T x^T   (lhsT = Wk chunk, rhs = x^T chunk)
  V   = x Wv       (lhsT = x^T chunk, rhs = Wv chunk)
Scores are computed transposed ([k, q], k on partitions) so softmax's
denominator comes from a ones-column appended to V (row 64 of the attention
output accumulator), and A^T is directly consumable by the A@V matmul.
exp() runs on the scalar engine with the 1/sqrt(dk) folded into its scale.
The normalized per-head outputs O^T are exactly the lhsT the output
projection wants, so no transposes are needed anywhere except on the input x.

Matmul operands are stored as fp16 (10-bit mantissa; measured end-to-end
absmax relative error ~4e-4): this is the true MAC path, so the PE
clock-gate can warm to 2.4 GHz and fast weight load applies. All
accumulation is fp32 in PSUM; softmax denominators/reciprocals are fp32.

The two heads' score matmuls share one [128,1024] PSUM tile and are pinned
adjacent via a scheduler dependency edge, so they stream through disjoint
PE row strips (0-63 / 64-127) concurrently; one exp() covers both. A@V
matmuls lag three k-tiles behind the scores so their exp() inputs are
always ready.
"""

from contextlib import ExitStack

import numpy as np

import concourse.bass as bass
import concourse.tile as tile
from concourse import bacc, mybir
from concourse.bass import _add_dep_helper
from concourse.bass_utils import run_bass_kernel_spmd

N_CORES = 8
B, S, D, H, DK = 2, 4096, 512, 8, 64
P = 128
NT_S = S // P                  # 32 sequence tiles
NT_D = D // P                  # 4 d-model chunks
QC = S // 512                  # 8 query chunks of 512
VW = 2 * 65                    # 130: per-k-tile width of the augmented V
F32 = mybir.dt.float32
F32R = mybir.dt.float32r
F16 = mybir.dt.float16
EXP = mybir.ActivationFunctionType.Exp

# "f16" (10 mantissa bits, 2.4 GHz MAC path + FWL), "f32r" (13 bits but
# pinned at the 1.2 GHz throttled clock), "f32" (exact, 4 cycles/row).
MM_DTYPE = "f16"
DTM = {"f32r": F32R, "f16": F16, "f32": F32}[MM_DTYPE]


def _emit(ctx: ExitStack, tc: tile.TileContext, io: dict):
    nc = tc.nc
    xb = io["xb"]
    wqp, wkp, wvp, wop = io["wqp"], io["wkp"], io["wvp"], io["wop"]
    bqp, bkp, bvp = io["bqp"], io["bkp"], io["bvp"]
    ident = io["ident"]
    out = io["out"]

    mm = nc.tensor.matmul

    # ---- pools ------------------------------------------------------------
    consts = ctx.enter_context(tc.tile_pool(name="consts", bufs=1))
    xt_pool = ctx.enter_context(tc.tile_pool(name="xt", bufs=1))
    qt_pool = ctx.enter_context(tc.tile_pool(name="qt", bufs=1))
    kt_pool = ctx.enter_context(tc.tile_pool(name="kt", bufs=1))
    v_pool = ctx.enter_context(tc.tile_pool(name="v", bufs=1))
    ot_pool = ctx.enter_context(tc.tile_pool(name="ot", bufs=2))
    w_pool = ctx.enter_context(tc.tile_pool(name="w", bufs=1))
    stg = ctx.enter_context(tc.tile_pool(name="stg", bufs=3))
    e_pool = ctx.enter_context(tc.tile_pool(name="e", bufs=8))
    rc_pool = ctx.enter_context(tc.tile_pool(name="rc", bufs=4))
    y_pool = ctx.enter_context(tc.tile_pool(name="y", bufs=3))
    # PSUM: shared [128,1024] pool (3 bufs x 2 banks) + attention
    # accumulators (2 banks). Projections use [0:512] slices of the pool.
    ps_pool = ctx.enter_context(tc.tile_pool(name="ps", bufs=3, space="PSUM"))
    o_pool = ctx.enter_context(tc.tile_pool(name="o", bufs=2, space="PSUM"))

    def psum1024(dt=F32):
        return ps_pool.tile([P, 1024], dt, tag="ps", name="ps")

    def psum512(dt=F32):
        return psum1024(dt)[:, 0:512]

    # ---- constants --------------------------------------------------------
    ident_sb = consts.tile([P, P], F32, tag="ident")
    nc.sync.dma_start(out=ident_sb[:], in_=ident[:])
    ones_f32 = consts.tile([P, 1], F32, tag="ones_f32")
    nc.vector.memset(ones_f32[:], 1.0)
    ones_sb = consts.tile([1, P], DTM, tag="ones")
    nc.vector.tensor_copy(out=ones_sb[:], in_=ones_f32[0:1, 0:1].broadcast_to([1, P]))
    # a f32 ones row living on partition 64 (denominator broadcast lhsT)
    ones64_sb = consts.tile([65, 64], F32, tag="ones64")
    nc.vector.memset(ones64_sb[64:65, :], 1.0)
    # per-partition bias columns for K^T/Q^T (fused into the PSUM->SBUF
    # copies); bv as a [1, 128] row for the rank-1 bias matmul.
    bkT = consts.tile([P, 1], F32, tag="bkT")
    nc.sync.dma_start(out=bkT[:], in_=bkp[:])
    bqT = consts.tile([P, 1], F32, tag="bqT")
    nc.sync.dma_start(out=bqT[:], in_=bqp[:])
    bv_st = consts.tile([1, P], F32, tag="bv_st")
    nc.sync.dma_start(out=bv_st[:], in_=bvp[:])
    bv_sb = consts.tile([1, P], DTM, tag="bv")
    nc.vector.tensor_copy(out=bv_sb[:], in_=bv_st[:])

    # per-core weight slices -> fp16 SBUF tiles
    def load_w(ap, rows, cols, tag):
        st = stg.tile([P, (rows // P) * cols], F32, tag="wstg")
        nc.sync.dma_start(
            out=st[:, :].rearrange("p (dc m) -> p dc m", dc=rows // P),
            in_=ap.rearrange("(dc p) m -> p dc m", p=P),
        )
        t = w_pool.tile([P, (rows // P) * cols], DTM, tag=tag)
        nc.vector.tensor_copy(out=t[:], in_=st[:])
        return t

    xT = xt_pool.tile([P, NT_D * S], DTM, tag="xT")

    # ---- stage A: x^T via PE transposes ----------------------------------
    with tc.tile_pool(name="xn", bufs=6) as xn_pool:
        for st in range(NT_S):
            xn = xn_pool.tile([P, D], F32, tag="xn")
            nc.sync.dma_start(out=xn[:], in_=xb[st * P:(st + 1) * P, :])
            tp = psum512()
            for dc in range(NT_D):
                nc.tensor.transpose(
                    tp[:, dc * P:(dc + 1) * P],
                    xn[:, dc * P:(dc + 1) * P],
                    ident_sb[:],
                )
            dst_ap = xT[:, :].rearrange("p (dc s) -> p dc s", dc=NT_D)
            nc.vector.tensor_copy(
                out=dst_ap[:, :, st * P:(st + 1) * P],
                in_=tp[:, :].rearrange("p (dc j) -> p dc j", dc=NT_D),
            )

    # ---- stage B: Q^T, K^T, V for the pair -------------------------------
    wq_sb = load_w(wqp, D, P, "wq")
    qt = qt_pool.tile([P, S], DTM, tag="QT")
    for sc in range(QC):
        ps = psum512()
        for dc in range(NT_D):
            mm(ps[:], wq_sb[:, dc * P:(dc + 1) * P],
               xT[:, dc * S + sc * 512:dc * S + (sc + 1) * 512],
               start=(dc == 0), stop=(dc == NT_D - 1))
        nc.vector.tensor_scalar_add(
            out=qt[:, sc * 512:(sc + 1) * 512], in0=ps[:], scalar1=bqT[:],
        )
    wk_sb = load_w(wkp, D, P, "wk")
    kt = kt_pool.tile([P, S], DTM, tag="KT")
    for sc in range(QC):
        ps = psum512()
        for dc in range(NT_D):
            mm(ps[:], wk_sb[:, dc * P:(dc + 1) * P],
               xT[:, dc * S + sc * 512:dc * S + (sc + 1) * 512],
               start=(dc == 0), stop=(dc == NT_D - 1))
        nc.vector.tensor_scalar_add(
            out=kt[:, sc * 512:(sc + 1) * 512], in0=ps[:], scalar1=bkT[:],
        )
    # V (2 heads) with a ones column per head:
    # vaug[:, kt*130 + hl*65 + (0..63)] = V[k-tile, head hl]; [.. + 64] = 1
    wv_sb = load_w(wvp, D, P, "wv")
    vaug = v_pool.tile([P, NT_S * VW], DTM, tag="vaug")
    nc.vector.tensor_copy(
        out=vaug[:, :].rearrange("p (t h e) -> p t h e",
                                 t=NT_S, h=2)[:, :, :, 64:65],
        in_=ones_f32[:, 0:1].broadcast_to([P, NT_S, 2, 1]),
    )
    for st in range(NT_S):
        ps = psum512()
        for dc in range(NT_D):
            mm(ps[:, 0:P], xT[:, dc * S + st * P:dc * S + (st + 1) * P],
               wv_sb[:, dc * P:(dc + 1) * P],
               start=(dc == 0), stop=False)
        mm(ps[:, 0:P], ones_sb[0:1, :], bv_sb[0:1, :],
           start=False, stop=True)
        dst = vaug[:, st * VW:(st + 1) * VW]
        dst = dst.rearrange("p (h e) -> p h e", h=2)[:, :, 0:64]
        nc.vector.tensor_copy(
            out=dst, in_=ps[:, 0:P].rearrange("p (h e) -> p h e", h=2)
        )

    # ---- stage C: attention ----------------------------------------------
    ot0 = ot_pool.tile([64, S], DTM, tag="OT")
    ot1 = ot_pool.tile([64, S], DTM, tag="OT")
    for qc in range(QC):
        qsl = slice(qc * 512, (qc + 1) * 512)
        o0 = o_pool.tile([65, 512], F32, tag="O")
        o1 = o_pool.tile([65, 512], F32, tag="O")

        def emit_av(ktile, ea, gate):
            st_ = ktile * VW
            fl = dict(start=(ktile == 0), stop=(ktile == NT_S - 1))
            i0 = mm(o0[:], vaug[:, st_ + 0 * 65:st_ + 0 * 65 + 65],
                    ea[:, 0:512], **fl)
            i1 = mm(o1[:], vaug[:, st_ + 1 * 65:st_ + 1 * 65 + 65],
                    ea[:, 512:1024], **fl)
            if gate is not None:
                # order A@V after the next score pair: keeps the paired
                # heads adjacent in the PE stream
                _add_dep_helper(i0.ins, gate.ins, sync=False,
                                reason="attn pipeline order")
                _add_dep_helper(i1.ins, gate.ins, sync=False,
                                reason="attn pipeline order")

        pending = []  # [(ktile, ea), ...] not yet AV-emitted
        for ktile in range(NT_S):
            ksl = slice(ktile * P, (ktile + 1) * P)
            # both heads' scores share one [128,1024] PSUM tile
            sp = psum1024()
            a = mm(sp[:, 0:512], kt[0:64, ksl], qt[0:64, qsl])
            b = mm(sp[:, 512:1024], kt[64:128, ksl], qt[64:128, qsl])
            # pin h64 right after h0: the pair streams through disjoint
            # PE row strips concurrently
            _add_dep_helper(b.ins, a.ins, sync=False, reason="pair order")
            # A@V lags three k-tiles behind the scores so its exp()
            # inputs are always long done.
            if len(pending) >= 3:
                pkt, pea = pending.pop(0)
                emit_av(pkt, pea, b)
            ea = e_pool.tile([P, 1024], DTM, tag="ea")
            nc.scalar.activation(ea[:], sp[:], EXP, scale=0.125)
            pending.append((ktile, ea))
        for pkt, pea in pending:
            emit_av(pkt, pea, None)
        # normalize: O[0:64] * (1 / O[64]) broadcast down. Copy O out of
        # PSUM immediately (frees the bank), then run the denominator
        # chain out of SBUF.
        for o_ps, ot in ((o0, ot0), (o1, ot1)):
            osb = rc_pool.tile([65, 512], F32, tag="osb")
            nc.vector.tensor_copy(out=osb[:], in_=o_ps[:])
            bc = psum512()
            mm(bc[0:64, :], ones64_sb[64:65, :], osb[64:65, :])
            rbc = rc_pool.tile([64, 512], F32, tag="rbc")
            nc.vector.reciprocal(out=rbc[:], in_=bc[0:64, :])
            nc.vector.tensor_mul(ot[:, qsl], osb[0:64, :], rbc[:])

    # ---- stage D: partial output projection Y = O_pair @ Wo_pair ---------
    # (no bias: the host adds bo once after summing the partials)
    wo_sb = []
    for hl in range(2):
        st = stg.tile([64, D], F32, tag="wostg")
        nc.sync.dma_start(out=st[:], in_=wop[hl * 64:(hl + 1) * 64, :])
        woh = w_pool.tile([64, D], DTM, tag=f"wo{hl}")
        nc.vector.tensor_copy(out=woh[:], in_=st[:])
        wo_sb.append(woh)
    for qt_i in range(S // P):
        ps = psum512()
        mm(ps[:], ot0[:, qt_i * P:(qt_i + 1) * P], wo_sb[0][:],
           start=True, stop=False)
        mm(ps[:], ot1[:, qt_i * P:(qt_i + 1) * P], wo_sb[1][:],
           start=False, stop=True)
        ysb = y_pool.tile([P, D], F32, tag="y")
        nc.vector.tensor_copy(out=ysb[:], in_=ps[:])
        nc.sync.dma_start(out=out[qt_i * P:(qt_i + 1) * P, :], in_=ysb[:])


def build():
    nc = bacc.Bacc("TRN2", target_bir_lowering=False, debug=False,
                   num_devices=N_CORES)
    io = {}
    for nm, shape in (("xb", [S, D]), ("wqp", [D, P]), ("wkp", [D, P]),
                      ("wvp", [D, P]), ("wop", [P, D]), ("bqp", [P, 1]),
                      ("bkp", [P, 1]), ("bvp", [1, P]), ("ident", [P, P])):
        io[nm] = nc.dram_tensor(nm, shape, F32, kind="ExternalInput").ap()
    io["out"] = nc.dram_tensor("out", [S, D], F32, kind="ExternalOutput").ap()
    with tile.TileContext(nc) as tc:
        with ExitStack() as ctx:
            _emit(ctx, tc, io)
    nc.compile()
    return nc


def make_in_maps(inputs):
    f = lambda a: np.ascontiguousarray(np.asarray(a, dtype=np.float32))
    x = f(inputs["x"])
    Wq, Wk, Wv, Wo = (f(inputs[k]) for k in ("Wq", "Wk", "Wv", "Wo"))
    bq, bk, bv = (f(inputs[k]).reshape(-1) for k in ("bq", "bk", "bv"))
    ident = np.eye(P, dtype=np.float32)
    in_maps = []
    for c in range(N_CORES):
        b, pr = c // 4, c % 4
        cs = slice(pr * P, (pr + 1) * P)
        in_maps.append({
            "xb": x[b],
            "wqp": f(Wq[:, cs]), "wkp": f(Wk[:, cs]), "wvp": f(Wv[:, cs]),
            "wop": f(Wo[cs, :]),
            "bqp": f(bq[cs]).reshape(P, 1), "bkp": f(bk[cs]).reshape(P, 1),
            "bvp": f(bv[cs]).reshape(1, P),
            "ident": ident,
        })
    return in_maps


_CACHE = {}
LAST_EXEC_NS = None


def run(inputs, trace=False):
    global LAST_EXEC_NS
    if "nc" not in _CACHE:
        _CACHE["nc"] = build()
    nc = _CACHE["nc"]
    kw = {}
    if trace:
        import sys, types
        if "antenv.axon_hooks" not in sys.modules:
            sys.path.insert(0, "/root/.axon_site")
            try:
                from trn_agent_boot.trn_boot import _ntff_profile_via_ctypes
                hook = _ntff_profile_via_ctypes("/opt/axon/libaxon_pjrt.so")
                mod = types.ModuleType("antenv.axon_hooks")
                mod.get_axon_ntff_profile_hook = lambda: hook
                mod.set_axon_ntff_profile_hook = lambda h: None
                sys.modules["antenv.axon_hooks"] = mod
            except Exception:
                pass
        kw = dict(trace=True, trace_cores=[0])
    res = run_bass_kernel_spmd(nc, make_in_maps(inputs),
                               core_ids=list(range(N_CORES)), **kw)
    if trace:
        LAST_EXEC_NS = res.exec_time_ns
    bo = np.asarray(inputs["bo"], np.float32).reshape(1, D)
    out = np.empty((B, S, D), np.float32)
    for b in range(B):
        acc = res.results[b * 4][ "out"].astype(np.float32).copy()
        for pr in range(1, 4):
            acc += res.results[b * 4 + pr]["out"]
        out[b] = acc + bo
    return out


def kernel(**inputs) -> np.ndarray:
    return run(inputs, trace=False)
